# revision 13
# baseline (speedup 1.0000x reference)
"""BsPINN forward MLP on 8 TRN2 NeuronCores (Bass/Tile), data-parallel over rows.

Network (per reference):
  h = 2*(X-lb)/(ub-lb)-1          [N,3]   (folded into W0/b0 on host)
  h = sin(h @ W0 + b0)            [N,1024]
  h = sin(h @ W1 + b1)            [N,1024] dense
  h = sin(h @ (W2*m2) + b2)       [N,1024] block-diag 2x(512x512)
  h = sin(h @ (W3*m3) + b3)       [N,1024] block-diag 4x(256x256)
  out = h @ W4 + b4               [N,1]

Design notes (v2; v1 measured 960us, see kernel_baseline.py):
  * Activations kept feature-major on chip (hT: features->partitions,
    rows->free); out_chunkT = W_chunk.T @ hT via nc.tensor.matmul, moving
    free dim 512 (one PSUM bank). Matmuls in float32r (fp32 RNE-rounded to
    11 mantissa bits on host), streaming 1 cycle/row.
  * Block-diagonal masks exploited by multiplying only in-block K-chunks
    (L2: 4 of 8, L3: 2 of 8) -- 60.3 GFLOP/core instead of 103.
  * L0 (K=4) thin matmuls are packed 2-per-PSUM-pair into DIFFERENT 32-row
    groups (tile_position (0,0) and (32,0), with x and W0 replicated at
    partition 32): the row-group LDWEIGHTS of the second MM overlaps the
    first MM's stream and the two MMs run concurrently, ~330ns per pair
    instead of 2x320ns serialized (v1 trace: thin LDW cannot hide behind a
    full-array MM).
  * Single merged per-row-tile schedule instead of v1's two phases, using
    the block-diag locality (L3 pair q needs only L2 pair q's output):
    [L1q0 L1q1 L1q2 L2q0 L0'p0 L2q1 L1q3 L0'p1 L2q2 L3q0 L0'p2 L2q3 L0'p3
     L3q1 L3q2 L3q3] keeps the ACT(sin) queue's duty even (~68%) so PSUM
    pair rotation never blocks the PE (v1's phase A was ACT-saturated).
  * Row-tile 0's L1 runs K-chunk-outer "waves" (8 MMs per arriving W1
    chunk, accumulating into 4 held PSUM pairs) so the PE consumes W1
    strictly in DMA arrival order; W1 chunks are spread over the scalar/
    sync HWDGE queues + the gpsimd SWDGE queue. Steady state starts ~20us
    vs ~34us in v1.
  * Sin on the scalar engine drains two PSUM banks per instruction; zero
    hidden biases (true here) let the bias ride W0's ones-row; a
    bias-general fallback program is compiled if biases are nonzero.
  * L4 (1024->1) runs as DVE per-partition multiply-accumulate plus a
    single f32r ones-matmul partition reduce, deferred one row-tile.
"""
import os
import numpy as np
import ml_dtypes

_BF16NP = ml_dtypes.bfloat16

try:  # run_bass_kernel_spmd(trace=True) imports this; absent in some images
    from antenv import axon_hooks as _axon_hooks  # noqa: F401
except ImportError:
    import sys
    import types
    _m = types.ModuleType("antenv.axon_hooks")
    _hook = [None]
    _m.set_axon_ntff_profile_hook = lambda h: _hook.__setitem__(0, h)
    _m.get_axon_ntff_profile_hook = lambda: _hook[0]
    sys.modules["antenv.axon_hooks"] = _m

import concourse.bass as bass
import concourse.tile as tile
from concourse import bacc, mybir
from concourse.bass_utils import run_bass_kernel_spmd

N_CORES = 8
N_FULL = 131072
R = N_FULL // N_CORES          # 16384 rows per core
NT = 512                       # matmul moving free dim (one PSUM bank, fp32)
RT = R // NT                   # 32 row tiles per core
NCH = 8                        # feature chunks (1024 / 128)

F32 = mybir.dt.float32
F32R = mybir.dt.float32r
BF16 = mybir.dt.bfloat16
SIN = mybir.ActivationFunctionType.Sin
IDENT = mybir.ActivationFunctionType.Identity

LAST_RESULTS = None
_PROGRAMS = {}


def _build_program(rt_count=RT, n_cores=N_CORES, act_pairs=True):
    nc = bacc.Bacc("TRN2", target_bir_lowering=False, debug=False,
                   num_devices=n_cores)

    xt_d = nc.dram_tensor("xt", [11, R], BF16, kind="ExternalInput").ap()
    w0_d = nc.dram_tensor("w0", [11, 1024], BF16, kind="ExternalInput").ap()
    w1_d = nc.dram_tensor("w1", [8, 128, 1024], BF16, kind="ExternalInput").ap()
    w2_d = nc.dram_tensor("w2", [8, 128, 512], BF16, kind="ExternalInput").ap()
    w3_d = nc.dram_tensor("w3", [8, 128, 256], BF16, kind="ExternalInput").ap()
    w4_d = nc.dram_tensor("w4", [128, 10], F32, kind="ExternalInput").ap()
    bias_d = nc.dram_tensor("bias", [128, 32], F32, kind="ExternalInput").ap()
    b4_d = nc.dram_tensor("b4", [1, 1], F32, kind="ExternalInput").ap()
    ones_d = nc.dram_tensor("onesr", [128, 2], BF16, kind="ExternalInput").ap()
    w4b_d = nc.dram_tensor("w4b", [128, 8], BF16, kind="ExternalInput").ap()
    o_d = nc.dram_tensor("o", [RT, NT], F32, kind="ExternalOutput").ap()

    with tile.TileContext(nc) as tc:
        with (
            tc.tile_pool(name="const", bufs=1) as cpool,
            tc.tile_pool(name="hbuf", bufs=2) as hpool,
            tc.tile_pool(name="xio", bufs=2) as xpool,
            tc.tile_pool(name="psum", bufs=4, space="PSUM") as ppool,
        ):
            # ---- one-time weight/bias setup ----------------------------
            # Weights arrive host-pre-rounded to the f32r grid and DMA
            # straight into f32r tiles. W0 (with its replica at partition
            # 32 for the row-group-packed L0) leads the scalar queue so
            # L0(rt0) can run at ~1us; W1 chunks are consumed in strict
            # arrival order by rt0's K-outer waves, spread over scalar/
            # sync/gpsimd so all 8 land by ~11us.
            # W0 row 3 holds b0 (the rhs carries a matching ones-row).
            w0r = cpool.tile([43, 1024], BF16, name="w0r", tag="w0r")
            nc.scalar.dma_start(out=w0r[0:11, :], in_=w0_d[:])
            nc.scalar.dma_start(out=w0r[32:43, :], in_=w0_d[:])

            xr0 = xpool.tile([43, NT], BF16, name="xr", tag="xr")
            nc.sync.dma_start(out=xr0[0:11, :], in_=xt_d[:, 0:NT])
            nc.sync.dma_start(out=xr0[32:43, :], in_=xt_d[:, 0:NT])

            w1r = [cpool.tile([128, 1024], BF16, name=f"w1r{kc}",
                              tag=f"w1r{kc}") for kc in range(NCH)]
            w2r = [cpool.tile([128, 512], BF16, name=f"w2r{kc}",
                              tag=f"w2r{kc}") for kc in range(NCH)]
            w3r = [cpool.tile([128, 256], BF16, name=f"w3r{kc}",
                              tag=f"w3r{kc}") for kc in range(NCH)]
            # w1 in 128KB half-chunks (lo: out cols 0-511 feeding mc 0-3
            # of wave kc, hi: cols 512-1023), spread over all three DMA
            # queues in wave-consumption order so rt0's waves are PE-bound
            # from ~9us. The scalar queue's DMAs (its queue blocks on each
            # transfer) all land before its first ACT is needed (~23us);
            # sync and gpsimd carry the rest.
            def w1lo(kc, eng):
                eng.dma_start(out=w1r[kc][:, 0:512], in_=w1_d[kc][:, 0:512])

            def w1hi(kc, eng):
                eng.dma_start(out=w1r[kc][:, 512:1024],
                              in_=w1_d[kc][:, 512:1024])

            # lo halves stream on sync, hi halves on gpsimd, one pair per
            # wave; the scalar queue carries nothing (each queued DMA
            # blocks it for the transfer time and the first sin is needed
            # at ~12us). Small/late tensors follow the w1 stream.
            for kc in range(NCH):
                w1lo(kc, nc.sync)
            for kc in range(NCH):
                w1hi(kc, nc.gpsimd)
            w4t = cpool.tile([128, 10], F32, name="w4t", tag="w4t")
            nc.gpsimd.dma_start(out=w4t[:], in_=w4_d[:])
            bt = cpool.tile([128, 32], F32, name="bt", tag="bt")
            nc.gpsimd.dma_start(out=bt[:], in_=bias_d[:])
            b4t = cpool.tile([1, 1], F32, name="b4t", tag="b4t")
            nc.gpsimd.dma_start(out=b4t[:], in_=b4_d[:])
            onesr = cpool.tile([128, 2], BF16, name="onesr", tag="onesr")
            nc.gpsimd.dma_start(out=onesr[:], in_=ones_d[:])
            w4b = cpool.tile([128, 8], BF16, name="w4b", tag="w4b")
            nc.gpsimd.dma_start(out=w4b[:], in_=w4b_d[:])
            # next row-tile's x, then the w2/w3 stream (all needed >20us)
            xr1 = xpool.tile([43, NT], BF16, name="xr", tag="xr")
            nc.sync.dma_start(out=xr1[0:11, :], in_=xt_d[:, NT:2 * NT])
            nc.sync.dma_start(out=xr1[32:43, :], in_=xt_d[:, NT:2 * NT])
            for kc in (0, 1, 2, 3):
                nc.sync.dma_start(out=w2r[kc][:], in_=w2_d[kc])
            for kc in (4, 5, 6, 7):
                nc.gpsimd.dma_start(out=w2r[kc][:], in_=w2_d[kc])
            for kc in (0, 1, 2, 3, 4, 5):
                nc.gpsimd.dma_start(out=w3r[kc][:], in_=w3_d[kc])
            for kc in (6, 7):
                nc.sync.dma_start(out=w3r[kc][:], in_=w3_d[kc])

            # PE warm-up: dep-free f32r matmuls bridge the first ~2us while
            # x/w0 DMA, keeping the HAM clock gate fed before real work.
            wmw = cpool.tile([128, 128], BF16, name="wmw", tag="wmw")
            nc.vector.memset(wmw[:], 0.0)
            wmx = cpool.tile([128, NT], BF16, name="wmx", tag="wmx")
            nc.vector.memset(wmx[:], 0.0)
            wmp = ppool.tile([128, NT], F32, name="wmp", tag="pt",
                             padded_shape=None)
            for i in range(2):
                nc.tensor.matmul(wmp[:, 0:NT], wmw[:], wmx[:],
                                 start=(i == 0), stop=(i == 1))

            PW = 2 * NT

            # ---- emitters ----------------------------------------------
            def emit_pair_l0(q, xr, bufs_):
                """L0 pair q: chunks 2q (rows 0:4, tile row-group 0) and
                2q+1 (rows 32:36, row-group 1). The two K=4 matmuls write
                the two banks of one PSUM pair and run concurrently; one
                wide Sin drains both."""
                hp = hpool.tile([128, 2 * NT], BF16, name=f"h1_{q}",
                                tag=f"h1_{q}", bufs=bufs_)
                pt = ppool.tile([128, PW], F32, name="pt", tag="pt")
                for half in range(2):
                    mc = 2 * q + half
                    rows = slice(0, 11) if half == 0 else slice(32, 43)
                    nc.tensor.matmul(
                        pt[:, half * NT:(half + 1) * NT],
                        lhsT=w0r[rows, 128 * mc:128 * (mc + 1)],
                        rhs=xr[rows, :], start=True, stop=True)
                nc.scalar.activation(hp[:], pt[:], SIN)
                return [hp[:, 0:NT], hp[:, NT:2 * NT]]

            def mm_l1(mc, j):
                kc = (mc + j) % NCH
                return dict(lhsT=w1r[kc][:, 128 * mc:128 * (mc + 1)],
                            rhs_idx=kc)

            def mm_l2(mc, j):
                b = mc // 4
                kcl = (mc + j) % 4
                return dict(lhsT=w2r[4 * b + kcl][:, (mc % 4) * 128:
                                                  (mc % 4) * 128 + 128],
                            rhs_idx=4 * b + kcl)

            def mm_l3(mc, j):
                bi = mc // 2
                kcl = (mc + j) % 2
                return dict(lhsT=w3r[2 * bi + kcl][:, (mc % 2) * 128:
                                                   (mc % 2) * 128 + 128],
                            rhs_idx=2 * bi + kcl)

            def emit_pair(lidx, q, nk, mm_args, hin, bufs_):
                """One 2-chunk group: both chunks share one 2-bank PSUM
                tile drained by a single wide Sin."""
                hp = hpool.tile([128, 2 * NT], BF16, name=f"h{lidx}_{q}",
                                tag=f"h{lidx}_{q}", bufs=bufs_)
                pt = ppool.tile([128, PW], F32, name="pt", tag="pt")
                for half in range(2):
                    mc = 2 * q + half
                    dst = pt[:, (half * NT):(half * NT) + NT]
                    for j in range(nk):
                        kw = mm_args(mc, j)
                        kc = kw.pop("rhs_idx")
                        nc.tensor.matmul(dst, rhs=hin[kc],
                                         start=(j == 0),
                                         stop=(j == nk - 1), **kw)
                nc.scalar.activation(hp[:], pt[:], SIN)
                return [hp[:, 0:NT], hp[:, NT:2 * NT]]

            def emit_l1_rt0(h1):
                """Row-tile 0's L1, K-chunk outer: 8 MMs per arriving W1
                chunk, accumulating into 4 simultaneously-held PSUM pairs.
                The PE consumes W1 in DMA arrival order."""
                pts = [ppool.tile([128, PW], F32, name="pt", tag="pt")
                       for _ in range(4)]
                for kc in range(NCH):
                    for mc in range(NCH):  # mc 0-3 need only the lo half
                        nc.tensor.matmul(
                            pts[mc // 2][:, (mc % 2) * NT:(mc % 2 + 1) * NT],
                            lhsT=w1r[kc][:, 128 * mc:128 * (mc + 1)],
                            rhs=h1[kc], start=(kc == 0), stop=(kc == NCH - 1))
                outs = []
                for q in range(4):
                    hp = hpool.tile([128, 2 * NT], BF16, name=f"h2_{q}",
                                    tag=f"h2_{q}", bufs=2)
                    nc.scalar.activation(hp[:], pts[q][:], SIN)
                    outs += [hp[:, 0:NT], hp[:, NT:2 * NT]]
                return outs

            def load_x(rt):
                cs = rt * NT
                xr = xpool.tile([43, NT], BF16, name="xr", tag="xr")
                nc.sync.dma_start(out=xr[0:11, :], in_=xt_d[:, cs:cs + NT])
                nc.sync.dma_start(out=xr[32:43, :], in_=xt_d[:, cs:cs + NT])
                return xr

            def flush_tail(pend):
                # partition-reduce of the deferred row-tile's L4 accumulator
                # (ones-matmul in f32r), bias, and store
                p_rt, acc = pend
                pt = ppool.tile([128, PW], F32, name="pt", tag="pt")
                nc.tensor.matmul(pt[0:2, 0:NT], onesr[:], acc[:],
                                 start=True, stop=True)
                ot = xpool.tile([1, NT], F32, name="ot", tag="ot")
                nc.vector.tensor_scalar_add(ot[:], pt[0:1, 0:NT], b4t[:])
                nc.gpsimd.dma_start(out=o_d[p_rt:p_rt + 1, :], in_=ot[0:1, :])

            # ---- generic-biases fallback (v1 structure, unpacked L0) ---
            def emit_layer_generic(lidx, nk, mm_args, hin, bufs_):
                outs = []
                for mc in range(NCH):
                    pt = ppool.tile([128, PW], F32, name="pt", tag="pt")
                    dst = pt[:, 0:NT]
                    for j in range(nk):
                        kw = mm_args(mc, j)
                        kc = kw.pop("rhs_idx")
                        nc.tensor.matmul(dst, rhs=hin[kc],
                                         start=(j == 0),
                                         stop=(j == nk - 1), **kw)
                    h = hpool.tile([128, NT], BF16, name=f"h{lidx}_{mc}",
                                   tag=f"h{lidx}_{mc}", bufs=bufs_)
                    if lidx == 1:
                        nc.scalar.activation(h[:], dst, SIN)
                    else:
                        c = 8 * (lidx - 1) + mc
                        nc.scalar.activation(h[:], dst, SIN,
                                             bias=bt[:, c:c + 1])
                    outs.append(h[:])
                return outs

            def mm_l0_generic(mc, j):
                return dict(lhsT=w0r[0:11, 128 * mc:128 * (mc + 1)],
                            rhs_idx=mc)

            if not act_pairs:
                h1 = emit_layer_generic(1, 1, mm_l0_generic,
                                        [xr0[0:11, :]] * NCH, 2)
                h2 = emit_layer_generic(2, NCH, mm_l1, h1, 2)
                pend = None
                for rt in range(rt_count):
                    xrn = load_x(rt + 1) if rt + 1 < rt_count else None
                    h3 = emit_layer_generic(3, 4, mm_l2, h2, 1)
                    h1n = (emit_layer_generic(1, 1, mm_l0_generic,
                                              [xrn[0:11, :]] * NCH, 2)
                           if xrn is not None else [])
                    h4 = emit_layer_generic(4, 2, mm_l3, h3, 2)
                    h2n = (emit_layer_generic(2, NCH, mm_l1, h1n, 2)
                           if h1n else [])
                    h2 = h2n
                    if pend is not None:
                        flush_tail(pend)
                        pend = None
                    acf = xpool.tile([128, NT], F32, name="acf", tag="acf",
                                     bufs=2)
                    ach = xpool.tile([128, NT], F32, name="ach", tag="ach",
                                     bufs=2)
                    nc.vector.tensor_scalar_mul(acf[:], h4[0],
                                                w4t[:, 0:1])
                    nc.vector.tensor_scalar_mul(ach[:], h4[4],
                                                w4t[:, 4:5])
                    for kc in (1, 2, 3):
                        nc.vector.scalar_tensor_tensor(
                            acf[:], h4[kc], w4t[:, kc:kc + 1],
                            acf[:], mybir.AluOpType.mult,
                            mybir.AluOpType.add)
                    for kc in (5, 6, 7):
                        nc.vector.scalar_tensor_tensor(
                            ach[:], h4[kc], w4t[:, kc:kc + 1],
                            ach[:], mybir.AluOpType.mult,
                            mybir.AluOpType.add)
                    acc = xpool.tile([128, NT], BF16, name="acc", tag="acc",
                                     bufs=2)
                    nc.vector.tensor_tensor(acc[:], acf[:], ach[:],
                                            mybir.AluOpType.add)
                    pend = (rt, acc)
                flush_tail(pend)
            else:
                # ---- merged steady-state schedule ----------------------
                h1 = []
                for q in range(4):
                    h1 += emit_pair_l0(q, xr0, 2)
                h2 = emit_l1_rt0(h1)
                xrn = xr1
                pend = None
                for rt in range(rt_count):
                    last = rt == rt_count - 1
                    h3, h4, h1n = [None] * 8, [None] * 8, []
                    if last:
                        acf = ach = None
                    else:
                        acf = xpool.tile([128, NT], F32, name="acf",
                                         tag="acf", bufs=2)
                        ach = xpool.tile([128, NT], F32, name="ach",
                                         tag="ach", bufs=2)

                    def l1q(q):
                        h2[2 * q:2 * q + 2] = emit_pair(2, q, NCH, mm_l1,
                                                        h1, 2)

                    def l2q(q):
                        h3[2 * q:2 * q + 2] = emit_pair(3, q, 4, mm_l2,
                                                        h2, 1)

                    def l3q(q):
                        h4[2 * q:2 * q + 2] = emit_pair(4, q, 2, mm_l3,
                                                        h3, 2)
                        if last:
                            return  # L4 runs as a PE burst after l3q(3)
                        dstt = acf if q < 2 else ach
                        for mc in (2 * q, 2 * q + 1):
                            if mc % 4 == 0:
                                nc.vector.tensor_scalar_mul(
                                    dstt[:], h4[mc],
                                    w4t[:, mc:mc + 1])
                            else:
                                nc.vector.scalar_tensor_tensor(
                                    dstt[:], h4[mc],
                                    w4t[:, mc:mc + 1], dstt[:],
                                    mybir.AluOpType.mult,
                                    mybir.AluOpType.add)

                    def l0p(q):
                        if xrn is not None:
                            h1n.extend(emit_pair_l0(q, xrn, 2))

                    # L1(rt) rebuilds h2 in place: l2q(0)/l2q(1) read only
                    # chunks 0-3 (block 0), written by l1q(0)/l1q(1) above
                    # them; l2q(2)/l2q(3) read 4-7, written by l1q(2)/(3).
                    if rt > 0:
                        l1q(0)
                        l1q(1)
                        if pend is not None:
                            flush_tail(pend)
                            pend = None
                        l1q(2)
                    l2q(0)
                    l0p(0)
                    l2q(1)
                    l3q(0)
                    if rt > 0:
                        l1q(3)
                    l0p(1)
                    l2q(2)
                    l3q(1)
                    l2q(3)
                    l0p(2)
                    l3q(2)
                    l0p(3)
                    l3q(3)

                    if not last:
                        acc = xpool.tile([128, NT], BF16, name="acc",
                                         tag="acc", bufs=2)
                        nc.vector.tensor_tensor(acc[:], acf[:], ach[:],
                                                mybir.AluOpType.add)
                        pend = (rt, acc)
                    h1 = h1n
                    xrn = load_x(rt + 2) if rt + 2 < rt_count else None

                # last row-tile's L4 as a short PE burst (accumulating
                # w4b partition-reduce matmuls) so no serial DVE chain is
                # exposed at the very end, then +b4 and store
                ptl = ppool.tile([128, PW], F32, name="ptl", tag="pt")
                for mc in range(NCH):
                    nc.tensor.matmul(ptl[0:1, 0:NT],
                                     lhsT=w4b[:, mc:mc + 1], rhs=h4[mc],
                                     start=(mc == 0), stop=(mc == NCH - 1))
                ot = xpool.tile([1, NT], F32, name="ot", tag="ot")
                nc.vector.tensor_scalar_add(ot[:], ptl[0:1, 0:NT], b4t[:])
                nc.gpsimd.dma_start(out=o_d[rt_count - 1:rt_count, :],
                                    in_=ot[0:1, :])

    nc.compile()
    return nc


def _get_program(act_pairs):
    key = act_pairs
    if key not in _PROGRAMS:
        _PROGRAMS[key] = _build_program(act_pairs=act_pairs)
    return _PROGRAMS[key]


def _rne11(x):
    """fp32 -> float32r grid: round-to-nearest-even keeping 11 mantissa bits
    (verified bit-identical to the on-chip f32r CAST)."""
    u = np.ascontiguousarray(x, np.float32).view(np.uint32).astype(np.uint64)
    bias = ((u >> 12) & 1) + (1 << 11) - 1
    return (((u + bias) >> 12) << 12).astype(np.uint32).view(np.float32)


def kernel(X, lb_X, ub_X, W0, b0, W1, b1, W2, b2, W3, b3, W4, b4):
    X = np.asarray(X, np.float32)
    lb = np.asarray(lb_X, np.float64)
    ub = np.asarray(ub_X, np.float64)
    W0 = np.asarray(W0, np.float64)
    b0 = np.asarray(b0, np.float64)

    # fold input normalization h = X*s + t into W0/b0:
    #   sin((X*s+t)@W0 + b0) = sin(X@(s[:,None]*W0) + (t@W0 + b0))
    # then expand K to 11 bf16 rows for full precision in bf16 matmuls:
    #   z = xh@W0h + xh@W0l + xl@W0h + bias_hi + bias_lo
    s = 2.0 / (ub - lb)
    t = -2.0 * lb / (ub - lb) - 1.0
    b0p = (b0 + t @ W0).astype(np.float32).reshape(1024)
    sW0 = (s[:, None] * W0).astype(np.float32)
    W0h = sW0.astype(_BF16NP)
    W0l = (sW0 - W0h.astype(np.float32)).astype(_BF16NP)
    bh = b0p.astype(_BF16NP)
    bl = (b0p - bh.astype(np.float32)).astype(_BF16NP)
    W0p = np.zeros((11, 1024), _BF16NP)
    W0p[0:3] = W0h
    W0p[3:6] = W0l
    W0p[6:9] = W0h
    W0p[9] = bh
    W0p[10] = bl

    W1 = np.asarray(W1, np.float32)
    W2 = np.asarray(W2, np.float32)
    W3 = np.asarray(W3, np.float32)
    W4 = np.asarray(W4, np.float32)
    b1 = np.asarray(b1, np.float32).reshape(1024)
    b2 = np.asarray(b2, np.float32).reshape(1024)
    b3 = np.asarray(b3, np.float32).reshape(1024)

    w1h = np.ascontiguousarray(W1.reshape(8, 128, 1024)).astype(_BF16NP)
    # W2: 2 blocks of 512x512 -> [4b+kcl] = W2[512b+128kcl:+128, 512b:+512]
    w2h = np.zeros((8, 128, 512), np.float32)
    for b in range(2):
        for kcl in range(4):
            w2h[4 * b + kcl] = W2[512 * b + 128 * kcl:512 * b + 128 * (kcl + 1),
                                  512 * b:512 * (b + 1)]
    # W3: 4 blocks of 256x256 -> [2bi+kcl] = W3[256bi+128kcl:+128, 256bi:+256]
    w3h = np.zeros((8, 128, 256), np.float32)
    for bi in range(4):
        for kcl in range(2):
            w3h[2 * bi + kcl] = W3[256 * bi + 128 * kcl:256 * bi + 128 * (kcl + 1),
                                   256 * bi:256 * (bi + 1)]
    # W4 [1024,1] -> [128,10]: col kc = W4[128kc:+128, 0]; cols 8-9 = ones
    # (stationary operand of the f32r partition-reduce matmul)
    w4h = np.ones((128, 10), np.float32)
    w4h[:, :8] = W4.reshape(8, 128).T
    # hidden-layer biases [128, 8] chunk-major columns (layers 1-3; layer 0's
    # bias is folded into the W0 ones-row)
    bh = np.zeros((128, 32), np.float32)
    for i, bb in enumerate([b1, b2, b3], start=1):
        bh[:, 8 * i:8 * (i + 1)] = bb.reshape(8, 128).T
    b4h = np.asarray(b4, np.float32).reshape(1, 1)

    w2h = w2h.astype(_BF16NP)
    w3h = w3h.astype(_BF16NP)
    act_pairs = not (b1.any() or b2.any() or b3.any())
    nc = _get_program(act_pairs)

    in_maps = []
    for c in range(N_CORES):
        xt = np.ones((11, R), _BF16NP)  # rows 9-10 = ones (bias rows)
        xc = X[c * R:(c + 1) * R].T
        xch = xc.astype(_BF16NP)
        xt[0:3] = xch
        xt[3:6] = xch
        xt[6:9] = (xc - xch.astype(np.float32)).astype(_BF16NP)
        in_maps.append({
            "xt": xt, "w0": W0p, "w1": w1h, "w2": w2h, "w3": w3h,
            "w4": w4h, "bias": bh, "b4": b4h,
            "onesr": np.ones((128, 2), _BF16NP),
            "w4b": w4h[:, :8].astype(_BF16NP),
        })

    trace = bool(int(os.environ.get("KERNEL_TRACE", "0")))
    res = run_bass_kernel_spmd(nc, in_maps, list(range(N_CORES)), trace=trace)
    global LAST_RESULTS
    LAST_RESULTS = res

    out = np.concatenate([res.results[c]["o"].reshape(R) for c in range(N_CORES)])
    return out.reshape(N_FULL, 1).astype(np.float32)


# revision 14
# speedup vs baseline: 1.0062x; 1.0062x over previous
"""BsPINN forward MLP on 8 TRN2 NeuronCores (Bass/Tile), data-parallel over rows.

Network (per reference):
  h = 2*(X-lb)/(ub-lb)-1          [N,3]   (folded into W0/b0 on host)
  h = sin(h @ W0 + b0)            [N,1024]
  h = sin(h @ W1 + b1)            [N,1024] dense
  h = sin(h @ (W2*m2) + b2)       [N,1024] block-diag 2x(512x512)
  h = sin(h @ (W3*m3) + b3)       [N,1024] block-diag 4x(256x256)
  out = h @ W4 + b4               [N,1]

Design notes (v2; v1 measured 960us, see kernel_baseline.py):
  * Activations kept feature-major on chip (hT: features->partitions,
    rows->free); out_chunkT = W_chunk.T @ hT via nc.tensor.matmul, moving
    free dim 512 (one PSUM bank). Matmuls in float32r (fp32 RNE-rounded to
    11 mantissa bits on host), streaming 1 cycle/row.
  * Block-diagonal masks exploited by multiplying only in-block K-chunks
    (L2: 4 of 8, L3: 2 of 8) -- 60.3 GFLOP/core instead of 103.
  * L0 (K=4) thin matmuls are packed 2-per-PSUM-pair into DIFFERENT 32-row
    groups (tile_position (0,0) and (32,0), with x and W0 replicated at
    partition 32): the row-group LDWEIGHTS of the second MM overlaps the
    first MM's stream and the two MMs run concurrently, ~330ns per pair
    instead of 2x320ns serialized (v1 trace: thin LDW cannot hide behind a
    full-array MM).
  * Single merged per-row-tile schedule instead of v1's two phases, using
    the block-diag locality (L3 pair q needs only L2 pair q's output):
    [L1q0 L1q1 L1q2 L2q0 L0'p0 L2q1 L1q3 L0'p1 L2q2 L3q0 L0'p2 L2q3 L0'p3
     L3q1 L3q2 L3q3] keeps the ACT(sin) queue's duty even (~68%) so PSUM
    pair rotation never blocks the PE (v1's phase A was ACT-saturated).
  * Row-tile 0's L1 runs K-chunk-outer "waves" (8 MMs per arriving W1
    chunk, accumulating into 4 held PSUM pairs) so the PE consumes W1
    strictly in DMA arrival order; W1 chunks are spread over the scalar/
    sync HWDGE queues + the gpsimd SWDGE queue. Steady state starts ~20us
    vs ~34us in v1.
  * Sin on the scalar engine drains two PSUM banks per instruction; zero
    hidden biases (true here) let the bias ride W0's ones-row; a
    bias-general fallback program is compiled if biases are nonzero.
  * L4 (1024->1) runs as DVE per-partition multiply-accumulate plus a
    single f32r ones-matmul partition reduce, deferred one row-tile.
"""
import os
import numpy as np
import ml_dtypes

_BF16NP = ml_dtypes.bfloat16

try:  # run_bass_kernel_spmd(trace=True) imports this; absent in some images
    from antenv import axon_hooks as _axon_hooks  # noqa: F401
except ImportError:
    import sys
    import types
    _m = types.ModuleType("antenv.axon_hooks")
    _hook = [None]
    _m.set_axon_ntff_profile_hook = lambda h: _hook.__setitem__(0, h)
    _m.get_axon_ntff_profile_hook = lambda: _hook[0]
    sys.modules["antenv.axon_hooks"] = _m

import concourse.bass as bass
import concourse.tile as tile
from concourse import bacc, mybir
from concourse.bass_utils import run_bass_kernel_spmd

N_CORES = 8
N_FULL = 131072
R = N_FULL // N_CORES          # 16384 rows per core
NT = 512                       # matmul moving free dim (one PSUM bank, fp32)
RT = R // NT                   # 32 row tiles per core
NCH = 8                        # feature chunks (1024 / 128)

F32 = mybir.dt.float32
F32R = mybir.dt.float32r
BF16 = mybir.dt.bfloat16
SIN = mybir.ActivationFunctionType.Sin
IDENT = mybir.ActivationFunctionType.Identity

LAST_RESULTS = None
_PROGRAMS = {}


def _build_program(rt_count=RT, n_cores=N_CORES, act_pairs=True):
    nc = bacc.Bacc("TRN2", target_bir_lowering=False, debug=False,
                   num_devices=n_cores)

    xt_d = nc.dram_tensor("xt", [11, R], BF16, kind="ExternalInput").ap()
    w0_d = nc.dram_tensor("w0", [11, 1024], BF16, kind="ExternalInput").ap()
    w1_d = nc.dram_tensor("w1", [8, 128, 1024], BF16, kind="ExternalInput").ap()
    w2_d = nc.dram_tensor("w2", [8, 128, 512], BF16, kind="ExternalInput").ap()
    w3_d = nc.dram_tensor("w3", [8, 128, 256], BF16, kind="ExternalInput").ap()
    w4_d = nc.dram_tensor("w4", [128, 10], F32, kind="ExternalInput").ap()
    bias_d = nc.dram_tensor("bias", [128, 32], F32, kind="ExternalInput").ap()
    b4_d = nc.dram_tensor("b4", [1, 1], F32, kind="ExternalInput").ap()
    ones_d = nc.dram_tensor("onesr", [128, 128], BF16, kind="ExternalInput").ap()
    w4b_d = nc.dram_tensor("w4b", [128, 1024], BF16, kind="ExternalInput").ap()
    o_d = nc.dram_tensor("o", [RT, NT], F32, kind="ExternalOutput").ap()

    with tile.TileContext(nc) as tc:
        with (
            tc.tile_pool(name="const", bufs=1) as cpool,
            tc.tile_pool(name="hbuf", bufs=2) as hpool,
            tc.tile_pool(name="xio", bufs=2) as xpool,
            tc.tile_pool(name="psum", bufs=4, space="PSUM") as ppool,
        ):
            # ---- one-time weight/bias setup ----------------------------
            # Weights arrive host-pre-rounded to the f32r grid and DMA
            # straight into f32r tiles. W0 (with its replica at partition
            # 32 for the row-group-packed L0) leads the scalar queue so
            # L0(rt0) can run at ~1us; W1 chunks are consumed in strict
            # arrival order by rt0's K-outer waves, spread over scalar/
            # sync/gpsimd so all 8 land by ~11us.
            # W0 row 3 holds b0 (the rhs carries a matching ones-row).
            # L0 runs as plain dense K=128 bf16 matmuls: weight rows 11-127
            # are zero, and the x tiles' rows 11-127 are zeroed once at
            # startup (the 2 rotation buffers keep their zeros; each
            # row-tile's DMA only rewrites rows 0-10). Zero weights x
            # finite garbage would be fine, but zero x zero avoids any
            # NaN-pattern risk.
            w0r = cpool.tile([128, 1024], BF16, name="w0r", tag="w0r")
            nc.vector.memset(w0r[:], 0.0)
            nc.scalar.dma_start(out=w0r[0:11, :], in_=w0_d[:])

            xr0 = xpool.tile([128, NT], BF16, name="xr", tag="xr")
            nc.vector.memset(xr0[:], 0.0)
            nc.sync.dma_start(out=xr0[0:11, :], in_=xt_d[:, 0:NT])

            w1r = [cpool.tile([128, 1024], BF16, name=f"w1r{kc}",
                              tag=f"w1r{kc}") for kc in range(NCH)]
            w2r = [cpool.tile([128, 512], BF16, name=f"w2r{kc}",
                              tag=f"w2r{kc}") for kc in range(NCH)]
            w3r = [cpool.tile([128, 256], BF16, name=f"w3r{kc}",
                              tag=f"w3r{kc}") for kc in range(NCH)]
            # w1 in 128KB half-chunks (lo: out cols 0-511 feeding mc 0-3
            # of wave kc, hi: cols 512-1023), spread over all three DMA
            # queues in wave-consumption order so rt0's waves are PE-bound
            # from ~9us. The scalar queue's DMAs (its queue blocks on each
            # transfer) all land before its first ACT is needed (~23us);
            # sync and gpsimd carry the rest.
            def w1lo(kc, eng):
                eng.dma_start(out=w1r[kc][:, 0:512], in_=w1_d[kc][:, 0:512])

            def w1hi(kc, eng):
                eng.dma_start(out=w1r[kc][:, 512:1024],
                              in_=w1_d[kc][:, 512:1024])

            # lo halves stream on sync, hi halves on gpsimd, one pair per
            # wave; the scalar queue carries nothing (each queued DMA
            # blocks it for the transfer time and the first sin is needed
            # at ~12us). Small/late tensors follow the w1 stream.
            for kc in range(NCH):
                w1lo(kc, nc.sync)
            for kc in range(NCH):
                w1hi(kc, nc.gpsimd)
            w4t = cpool.tile([128, 10], F32, name="w4t", tag="w4t")
            nc.gpsimd.dma_start(out=w4t[:], in_=w4_d[:])
            bt = cpool.tile([128, 32], F32, name="bt", tag="bt")
            nc.gpsimd.dma_start(out=bt[:], in_=bias_d[:])
            b4t = cpool.tile([1, 1], F32, name="b4t", tag="b4t")
            nc.gpsimd.dma_start(out=b4t[:], in_=b4_d[:])
            onesr = cpool.tile([128, 128], BF16, name="onesr", tag="onesr")
            nc.gpsimd.dma_start(out=onesr[:], in_=ones_d[:])
            w4b = cpool.tile([128, 1024], BF16, name="w4b", tag="w4b")
            nc.gpsimd.dma_start(out=w4b[:], in_=w4b_d[:])
            # next row-tile's x, then the w2/w3 stream (all needed >20us)
            xr1 = xpool.tile([128, NT], BF16, name="xr", tag="xr")
            nc.vector.memset(xr1[:], 0.0)
            nc.sync.dma_start(out=xr1[0:11, :], in_=xt_d[:, NT:2 * NT])
            for kc in (0, 1, 2, 3):
                nc.sync.dma_start(out=w2r[kc][:], in_=w2_d[kc])
            for kc in (4, 5, 6, 7):
                nc.gpsimd.dma_start(out=w2r[kc][:], in_=w2_d[kc])
            for kc in (0, 1, 2, 3, 4, 5):
                nc.gpsimd.dma_start(out=w3r[kc][:], in_=w3_d[kc])
            for kc in (6, 7):
                nc.sync.dma_start(out=w3r[kc][:], in_=w3_d[kc])

            # PE warm-up: dep-free f32r matmuls bridge the first ~2us while
            # x/w0 DMA, keeping the HAM clock gate fed before real work.
            wmw = cpool.tile([128, 128], BF16, name="wmw", tag="wmw")
            nc.vector.memset(wmw[:], 0.0)
            wmx = cpool.tile([128, NT], BF16, name="wmx", tag="wmx")
            nc.vector.memset(wmx[:], 0.0)
            wmp = ppool.tile([128, NT], F32, name="wmp", tag="pt",
                             padded_shape=None)
            for i in range(2):
                nc.tensor.matmul(wmp[:, 0:NT], wmw[:], wmx[:],
                                 start=(i == 0), stop=(i == 1))

            PW = 2 * NT

            # ---- emitters ----------------------------------------------
            def mm_l0(mc, j):
                return dict(lhsT=w0r[:, 128 * mc:128 * (mc + 1)], rhs_idx=0)

            def emit_pair_l0(q, xr, bufs_):
                """L0 pair q: two dense K=128 bf16 matmuls (zero-padded
                weight rows) — same stream shape as every other layer."""
                return emit_pair(1, q, 1, mm_l0, [xr[:]], bufs_)

            def mm_l1(mc, j):
                kc = (mc + j) % NCH
                return dict(lhsT=w1r[kc][:, 128 * mc:128 * (mc + 1)],
                            rhs_idx=kc)

            def mm_l2(mc, j):
                b = mc // 4
                kcl = (mc + j) % 4
                return dict(lhsT=w2r[4 * b + kcl][:, (mc % 4) * 128:
                                                  (mc % 4) * 128 + 128],
                            rhs_idx=4 * b + kcl)

            def mm_l3(mc, j):
                bi = mc // 2
                kcl = (mc + j) % 2
                return dict(lhsT=w3r[2 * bi + kcl][:, (mc % 2) * 128:
                                                   (mc % 2) * 128 + 128],
                            rhs_idx=2 * bi + kcl)

            def emit_pair(lidx, q, nk, mm_args, hin, bufs_):
                """One 2-chunk group: both chunks share one 2-bank PSUM
                tile drained by a single wide Sin."""
                hp = hpool.tile([128, 2 * NT], BF16, name=f"h{lidx}_{q}",
                                tag=f"h{lidx}_{q}", bufs=bufs_)
                pt = ppool.tile([128, PW], F32, name="pt", tag="pt")
                for half in range(2):
                    mc = 2 * q + half
                    dst = pt[:, (half * NT):(half * NT) + NT]
                    for j in range(nk):
                        kw = mm_args(mc, j)
                        kc = kw.pop("rhs_idx")
                        nc.tensor.matmul(dst, rhs=hin[kc],
                                         start=(j == 0),
                                         stop=(j == nk - 1), **kw)
                nc.scalar.activation(hp[:], pt[:], SIN)
                return [hp[:, 0:NT], hp[:, NT:2 * NT]]

            def emit_l1_rt0(h1):
                """Row-tile 0's L1, K-chunk outer: 8 MMs per arriving W1
                chunk, accumulating into 4 simultaneously-held PSUM pairs.
                The PE consumes W1 in DMA arrival order."""
                pts = [ppool.tile([128, PW], F32, name="pt", tag="pt")
                       for _ in range(4)]
                for kc in range(NCH):
                    for mc in range(NCH):  # mc 0-3 need only the lo half
                        nc.tensor.matmul(
                            pts[mc // 2][:, (mc % 2) * NT:(mc % 2 + 1) * NT],
                            lhsT=w1r[kc][:, 128 * mc:128 * (mc + 1)],
                            rhs=h1[kc], start=(kc == 0), stop=(kc == NCH - 1))
                outs = []
                for q in range(4):
                    hp = hpool.tile([128, 2 * NT], BF16, name=f"h2_{q}",
                                    tag=f"h2_{q}", bufs=2)
                    nc.scalar.activation(hp[:], pts[q][:], SIN)
                    outs += [hp[:, 0:NT], hp[:, NT:2 * NT]]
                return outs

            def load_x(rt):
                cs = rt * NT
                xr = xpool.tile([128, NT], BF16, name="xr", tag="xr")
                nc.sync.dma_start(out=xr[0:11, :], in_=xt_d[:, cs:cs + NT])
                return xr

            def flush_tail(pend):
                # partition-reduce of the deferred row-tile's L4 accumulator
                # (ones-matmul in f32r), bias, and store
                p_rt, acc = pend
                pt = ppool.tile([128, PW], F32, name="pt", tag="pt")
                nc.tensor.matmul(pt[:, 0:NT], onesr[:], acc[:],
                                 start=True, stop=True)
                ot = xpool.tile([1, NT], F32, name="ot", tag="ot")
                nc.vector.tensor_scalar_add(ot[:], pt[0:1, 0:NT], b4t[:])
                nc.gpsimd.dma_start(out=o_d[p_rt:p_rt + 1, :], in_=ot[0:1, :])

            # ---- generic-biases fallback (v1 structure, unpacked L0) ---
            def emit_layer_generic(lidx, nk, mm_args, hin, bufs_):
                outs = []
                for mc in range(NCH):
                    pt = ppool.tile([128, PW], F32, name="pt", tag="pt")
                    dst = pt[:, 0:NT]
                    for j in range(nk):
                        kw = mm_args(mc, j)
                        kc = kw.pop("rhs_idx")
                        nc.tensor.matmul(dst, rhs=hin[kc],
                                         start=(j == 0),
                                         stop=(j == nk - 1), **kw)
                    h = hpool.tile([128, NT], BF16, name=f"h{lidx}_{mc}",
                                   tag=f"h{lidx}_{mc}", bufs=bufs_)
                    if lidx == 1:
                        nc.scalar.activation(h[:], dst, SIN)
                    else:
                        c = 8 * (lidx - 1) + mc
                        nc.scalar.activation(h[:], dst, SIN,
                                             bias=bt[:, c:c + 1])
                    outs.append(h[:])
                return outs

            def mm_l0_generic(mc, j):
                return dict(lhsT=w0r[:, 128 * mc:128 * (mc + 1)],
                            rhs_idx=mc)

            if not act_pairs:
                h1 = emit_layer_generic(1, 1, mm_l0_generic,
                                        [xr0[:]] * NCH, 2)
                h2 = emit_layer_generic(2, NCH, mm_l1, h1, 2)
                pend = None
                for rt in range(rt_count):
                    xrn = load_x(rt + 1) if rt + 1 < rt_count else None
                    h3 = emit_layer_generic(3, 4, mm_l2, h2, 1)
                    h1n = (emit_layer_generic(1, 1, mm_l0_generic,
                                              [xrn[:]] * NCH, 2)
                           if xrn is not None else [])
                    h4 = emit_layer_generic(4, 2, mm_l3, h3, 2)
                    h2n = (emit_layer_generic(2, NCH, mm_l1, h1n, 2)
                           if h1n else [])
                    h2 = h2n
                    if pend is not None:
                        flush_tail(pend)
                        pend = None
                    acf = xpool.tile([128, NT], F32, name="acf", tag="acf",
                                     bufs=2)
                    ach = xpool.tile([128, NT], F32, name="ach", tag="ach",
                                     bufs=2)
                    nc.vector.tensor_scalar_mul(acf[:], h4[0],
                                                w4t[:, 0:1])
                    nc.vector.tensor_scalar_mul(ach[:], h4[4],
                                                w4t[:, 4:5])
                    for kc in (1, 2, 3):
                        nc.vector.scalar_tensor_tensor(
                            acf[:], h4[kc], w4t[:, kc:kc + 1],
                            acf[:], mybir.AluOpType.mult,
                            mybir.AluOpType.add)
                    for kc in (5, 6, 7):
                        nc.vector.scalar_tensor_tensor(
                            ach[:], h4[kc], w4t[:, kc:kc + 1],
                            ach[:], mybir.AluOpType.mult,
                            mybir.AluOpType.add)
                    acc = xpool.tile([128, NT], BF16, name="acc", tag="acc",
                                     bufs=2)
                    nc.vector.tensor_tensor(acc[:], acf[:], ach[:],
                                            mybir.AluOpType.add)
                    pend = (rt, acc)
                flush_tail(pend)
            else:
                # ---- merged steady-state schedule ----------------------
                h1 = []
                for q in range(4):
                    h1 += emit_pair_l0(q, xr0, 2)
                h2 = emit_l1_rt0(h1)
                xrn = xr1
                pend = None
                for rt in range(rt_count):
                    last = rt == rt_count - 1
                    h3, h4, h1n = [None] * 8, [None] * 8, []
                    if last:
                        acf = ach = None
                    else:
                        acf = xpool.tile([128, NT], F32, name="acf",
                                         tag="acf", bufs=2)
                        ach = xpool.tile([128, NT], F32, name="ach",
                                         tag="ach", bufs=2)

                    def l1q(q):
                        h2[2 * q:2 * q + 2] = emit_pair(2, q, NCH, mm_l1,
                                                        h1, 2)

                    def l2q(q):
                        h3[2 * q:2 * q + 2] = emit_pair(3, q, 4, mm_l2,
                                                        h2, 1)

                    def l3q(q):
                        h4[2 * q:2 * q + 2] = emit_pair(4, q, 2, mm_l3,
                                                        h3, 2)
                        if last:
                            return  # L4 runs as a PE burst after l3q(3)
                        dstt = acf if q < 2 else ach
                        for mc in (2 * q, 2 * q + 1):
                            if mc % 4 == 0:
                                nc.vector.tensor_scalar_mul(
                                    dstt[:], h4[mc],
                                    w4t[:, mc:mc + 1])
                            else:
                                nc.vector.scalar_tensor_tensor(
                                    dstt[:], h4[mc],
                                    w4t[:, mc:mc + 1], dstt[:],
                                    mybir.AluOpType.mult,
                                    mybir.AluOpType.add)

                    def l0p(q):
                        if xrn is not None:
                            h1n.extend(emit_pair_l0(q, xrn, 2))

                    # L1(rt) rebuilds h2 in place: l2q(0)/l2q(1) read only
                    # chunks 0-3 (block 0), written by l1q(0)/l1q(1) above
                    # them; l2q(2)/l2q(3) read 4-7, written by l1q(2)/(3).
                    if rt > 0:
                        l1q(0)
                        l1q(1)
                        if pend is not None:
                            flush_tail(pend)
                            pend = None
                        l1q(2)
                    l2q(0)
                    l0p(0)
                    l2q(1)
                    l3q(0)
                    if rt > 0:
                        l1q(3)
                    l0p(1)
                    l3q(1)
                    l2q(2)
                    l2q(3)
                    l0p(2)
                    l3q(2)
                    l0p(3)
                    l3q(3)

                    if not last:
                        acc = xpool.tile([128, NT], BF16, name="acc",
                                         tag="acc", bufs=2)
                        nc.vector.tensor_tensor(acc[:], acf[:], ach[:],
                                                mybir.AluOpType.add)
                        pend = (rt, acc)
                    h1 = h1n
                    xrn = load_x(rt + 2) if rt + 2 < rt_count else None

                # last row-tile's L4 as a short PE burst (accumulating
                # w4b partition-reduce matmuls) so no serial DVE chain is
                # exposed at the very end, then +b4 and store
                ptl = ppool.tile([128, PW], F32, name="ptl", tag="pt")
                for mc in range(NCH):
                    nc.tensor.matmul(ptl[:, 0:NT],
                                     lhsT=w4b[:, 128 * mc:128 * (mc + 1)],
                                     rhs=h4[mc],
                                     start=(mc == 0), stop=(mc == NCH - 1))
                ot = xpool.tile([1, NT], F32, name="ot", tag="ot")
                nc.vector.tensor_scalar_add(ot[:], ptl[0:1, 0:NT], b4t[:])
                nc.gpsimd.dma_start(out=o_d[rt_count - 1:rt_count, :],
                                    in_=ot[0:1, :])

    nc.compile()
    return nc


def _get_program(act_pairs):
    key = act_pairs
    if key not in _PROGRAMS:
        _PROGRAMS[key] = _build_program(act_pairs=act_pairs)
    return _PROGRAMS[key]


def _rne11(x):
    """fp32 -> float32r grid: round-to-nearest-even keeping 11 mantissa bits
    (verified bit-identical to the on-chip f32r CAST)."""
    u = np.ascontiguousarray(x, np.float32).view(np.uint32).astype(np.uint64)
    bias = ((u >> 12) & 1) + (1 << 11) - 1
    return (((u + bias) >> 12) << 12).astype(np.uint32).view(np.float32)


def kernel(X, lb_X, ub_X, W0, b0, W1, b1, W2, b2, W3, b3, W4, b4):
    X = np.asarray(X, np.float32)
    lb = np.asarray(lb_X, np.float64)
    ub = np.asarray(ub_X, np.float64)
    W0 = np.asarray(W0, np.float64)
    b0 = np.asarray(b0, np.float64)

    # fold input normalization h = X*s + t into W0/b0:
    #   sin((X*s+t)@W0 + b0) = sin(X@(s[:,None]*W0) + (t@W0 + b0))
    # then expand K to 11 bf16 rows for full precision in bf16 matmuls:
    #   z = xh@W0h + xh@W0l + xl@W0h + bias_hi + bias_lo
    s = 2.0 / (ub - lb)
    t = -2.0 * lb / (ub - lb) - 1.0
    b0p = (b0 + t @ W0).astype(np.float32).reshape(1024)
    sW0 = (s[:, None] * W0).astype(np.float32)
    W0h = sW0.astype(_BF16NP)
    W0l = (sW0 - W0h.astype(np.float32)).astype(_BF16NP)
    bh = b0p.astype(_BF16NP)
    bl = (b0p - bh.astype(np.float32)).astype(_BF16NP)
    W0p = np.zeros((11, 1024), _BF16NP)
    W0p[0:3] = W0h
    W0p[3:6] = W0l
    W0p[6:9] = W0h
    W0p[9] = bh
    W0p[10] = bl

    W1 = np.asarray(W1, np.float32)
    W2 = np.asarray(W2, np.float32)
    W3 = np.asarray(W3, np.float32)
    W4 = np.asarray(W4, np.float32)
    b1 = np.asarray(b1, np.float32).reshape(1024)
    b2 = np.asarray(b2, np.float32).reshape(1024)
    b3 = np.asarray(b3, np.float32).reshape(1024)

    w1h = np.ascontiguousarray(W1.reshape(8, 128, 1024)).astype(_BF16NP)
    # W2: 2 blocks of 512x512 -> [4b+kcl] = W2[512b+128kcl:+128, 512b:+512]
    w2h = np.zeros((8, 128, 512), np.float32)
    for b in range(2):
        for kcl in range(4):
            w2h[4 * b + kcl] = W2[512 * b + 128 * kcl:512 * b + 128 * (kcl + 1),
                                  512 * b:512 * (b + 1)]
    # W3: 4 blocks of 256x256 -> [2bi+kcl] = W3[256bi+128kcl:+128, 256bi:+256]
    w3h = np.zeros((8, 128, 256), np.float32)
    for bi in range(4):
        for kcl in range(2):
            w3h[2 * bi + kcl] = W3[256 * bi + 128 * kcl:256 * bi + 128 * (kcl + 1),
                                   256 * bi:256 * (bi + 1)]
    # W4 [1024,1] -> [128,10]: col kc = W4[128kc:+128, 0]; cols 8-9 = ones
    # (stationary operand of the f32r partition-reduce matmul)
    w4h = np.ones((128, 10), np.float32)
    w4h[:, :8] = W4.reshape(8, 128).T
    # hidden-layer biases [128, 8] chunk-major columns (layers 1-3; layer 0's
    # bias is folded into the W0 ones-row)
    bh = np.zeros((128, 32), np.float32)
    for i, bb in enumerate([b1, b2, b3], start=1):
        bh[:, 8 * i:8 * (i + 1)] = bb.reshape(8, 128).T
    b4h = np.asarray(b4, np.float32).reshape(1, 1)

    w2h = w2h.astype(_BF16NP)
    w3h = w3h.astype(_BF16NP)
    # partition-reduce stationaries, zero-padded to full 128-wide weights
    # so the reduce matmuls keep the dense LDW/MM pipeline shape
    onesh = np.zeros((128, 128), _BF16NP)
    onesh[:, 0] = 1
    w4bp = np.zeros((128, 1024), np.float32)
    for mc in range(8):
        w4bp[:, 128 * mc] = w4h[:, mc]
    w4bp = w4bp.astype(_BF16NP)
    act_pairs = not (b1.any() or b2.any() or b3.any())
    nc = _get_program(act_pairs)

    in_maps = []
    for c in range(N_CORES):
        xt = np.ones((11, R), _BF16NP)  # rows 9-10 = ones (bias rows)
        xc = X[c * R:(c + 1) * R].T
        xch = xc.astype(_BF16NP)
        xt[0:3] = xch
        xt[3:6] = xch
        xt[6:9] = (xc - xch.astype(np.float32)).astype(_BF16NP)
        in_maps.append({
            "xt": xt, "w0": W0p, "w1": w1h, "w2": w2h, "w3": w3h,
            "w4": w4h, "bias": bh, "b4": b4h,
            "onesr": onesh, "w4b": w4bp,
        })

    trace = bool(int(os.environ.get("KERNEL_TRACE", "0")))
    res = run_bass_kernel_spmd(nc, in_maps, list(range(N_CORES)), trace=trace)
    global LAST_RESULTS
    LAST_RESULTS = res

    out = np.concatenate([res.results[c]["o"].reshape(R) for c in range(N_CORES)])
    return out.reshape(N_FULL, 1).astype(np.float32)


# revision 15
# speedup vs baseline: 1.0161x; 1.0099x over previous
"""BsPINN forward MLP on 8 TRN2 NeuronCores (Bass/Tile), data-parallel over rows.

Network (per reference):
  h = 2*(X-lb)/(ub-lb)-1          [N,3]   (folded into W0/b0 on host)
  h = sin(h @ W0 + b0)            [N,1024]
  h = sin(h @ W1 + b1)            [N,1024] dense
  h = sin(h @ (W2*m2) + b2)       [N,1024] block-diag 2x(512x512)
  h = sin(h @ (W3*m3) + b3)       [N,1024] block-diag 4x(256x256)
  out = h @ W4 + b4               [N,1]

Design notes (v2; v1 measured 960us, see kernel_baseline.py):
  * Activations kept feature-major on chip (hT: features->partitions,
    rows->free); out_chunkT = W_chunk.T @ hT via nc.tensor.matmul, moving
    free dim 512 (one PSUM bank). Matmuls in float32r (fp32 RNE-rounded to
    11 mantissa bits on host), streaming 1 cycle/row.
  * Block-diagonal masks exploited by multiplying only in-block K-chunks
    (L2: 4 of 8, L3: 2 of 8) -- 60.3 GFLOP/core instead of 103.
  * L0 (K=4) thin matmuls are packed 2-per-PSUM-pair into DIFFERENT 32-row
    groups (tile_position (0,0) and (32,0), with x and W0 replicated at
    partition 32): the row-group LDWEIGHTS of the second MM overlaps the
    first MM's stream and the two MMs run concurrently, ~330ns per pair
    instead of 2x320ns serialized (v1 trace: thin LDW cannot hide behind a
    full-array MM).
  * Single merged per-row-tile schedule instead of v1's two phases, using
    the block-diag locality (L3 pair q needs only L2 pair q's output):
    [L1q0 L1q1 L1q2 L2q0 L0'p0 L2q1 L1q3 L0'p1 L2q2 L3q0 L0'p2 L2q3 L0'p3
     L3q1 L3q2 L3q3] keeps the ACT(sin) queue's duty even (~68%) so PSUM
    pair rotation never blocks the PE (v1's phase A was ACT-saturated).
  * Row-tile 0's L1 runs K-chunk-outer "waves" (8 MMs per arriving W1
    chunk, accumulating into 4 held PSUM pairs) so the PE consumes W1
    strictly in DMA arrival order; W1 chunks are spread over the scalar/
    sync HWDGE queues + the gpsimd SWDGE queue. Steady state starts ~20us
    vs ~34us in v1.
  * Sin on the scalar engine drains two PSUM banks per instruction; zero
    hidden biases (true here) let the bias ride W0's ones-row; a
    bias-general fallback program is compiled if biases are nonzero.
  * L4 (1024->1) runs as DVE per-partition multiply-accumulate plus a
    single f32r ones-matmul partition reduce, deferred one row-tile.
"""
import os
import numpy as np
import ml_dtypes

_BF16NP = ml_dtypes.bfloat16

try:  # run_bass_kernel_spmd(trace=True) imports this; absent in some images
    from antenv import axon_hooks as _axon_hooks  # noqa: F401
except ImportError:
    import sys
    import types
    _m = types.ModuleType("antenv.axon_hooks")
    _hook = [None]
    _m.set_axon_ntff_profile_hook = lambda h: _hook.__setitem__(0, h)
    _m.get_axon_ntff_profile_hook = lambda: _hook[0]
    sys.modules["antenv.axon_hooks"] = _m

import concourse.bass as bass
import concourse.tile as tile
from concourse import bacc, mybir
from concourse.bass_utils import run_bass_kernel_spmd
from concourse import bass_isa

N_CORES = 8
N_FULL = 131072
R = N_FULL // N_CORES          # 16384 rows per core
NT = 512                       # matmul moving free dim (one PSUM bank, fp32)
RT = R // NT                   # 32 row tiles per core
NCH = 8                        # feature chunks (1024 / 128)

F32 = mybir.dt.float32
F32R = mybir.dt.float32r
BF16 = mybir.dt.bfloat16
SIN = mybir.ActivationFunctionType.Sin
IDENT = mybir.ActivationFunctionType.Identity

LAST_RESULTS = None
_PROGRAMS = {}


def _build_program(rt_count=RT, n_cores=N_CORES, act_pairs=True):
    nc = bacc.Bacc("TRN2", target_bir_lowering=False, debug=False,
                   num_devices=n_cores)

    xt_d = nc.dram_tensor("xt", [11, R], BF16, kind="ExternalInput").ap()
    w0_d = nc.dram_tensor("w0", [11, 1024], BF16, kind="ExternalInput").ap()
    w1_d = nc.dram_tensor("w1", [8, 128, 1024], BF16, kind="ExternalInput").ap()
    w2_d = nc.dram_tensor("w2", [8, 128, 512], BF16, kind="ExternalInput").ap()
    w3_d = nc.dram_tensor("w3", [8, 128, 256], BF16, kind="ExternalInput").ap()
    w4_d = nc.dram_tensor("w4", [128, 10], F32, kind="ExternalInput").ap()
    bias_d = nc.dram_tensor("bias", [128, 32], F32, kind="ExternalInput").ap()
    b4_d = nc.dram_tensor("b4", [1, 1], F32, kind="ExternalInput").ap()
    ones_d = nc.dram_tensor("onesr", [128, 128], BF16, kind="ExternalInput").ap()
    w4b_d = nc.dram_tensor("w4b", [128, 1024], BF16, kind="ExternalInput").ap()
    o_d = nc.dram_tensor("o", [RT, NT], F32, kind="ExternalOutput").ap()

    with tile.TileContext(nc) as tc:
        with (
            tc.tile_pool(name="const", bufs=1) as cpool,
            tc.tile_pool(name="hbuf", bufs=2) as hpool,
            tc.tile_pool(name="xio", bufs=2) as xpool,
            tc.tile_pool(name="psum", bufs=4, space="PSUM") as ppool,
        ):
            # ---- one-time weight/bias setup ----------------------------
            # Weights arrive host-pre-rounded to the f32r grid and DMA
            # straight into f32r tiles. W0 (with its replica at partition
            # 32 for the row-group-packed L0) leads the scalar queue so
            # L0(rt0) can run at ~1us; W1 chunks are consumed in strict
            # arrival order by rt0's K-outer waves, spread over scalar/
            # sync/gpsimd so all 8 land by ~11us.
            # W0 row 3 holds b0 (the rhs carries a matching ones-row).
            # L0 runs as plain dense K=128 bf16 matmuls: weight rows 11-127
            # are zero, and the x tiles' rows 11-127 are zeroed once at
            # startup (the 2 rotation buffers keep their zeros; each
            # row-tile's DMA only rewrites rows 0-10). Zero weights x
            # finite garbage would be fine, but zero x zero avoids any
            # NaN-pattern risk.
            w0r = cpool.tile([128, 1024], BF16, name="w0r", tag="w0r")
            nc.vector.memset(w0r[:], 0.0)
            nc.scalar.dma_start(out=w0r[0:11, :], in_=w0_d[:])

            xr0 = xpool.tile([128, NT], BF16, name="xr", tag="xr")
            nc.vector.memset(xr0[:], 0.0)
            nc.sync.dma_start(out=xr0[0:11, :], in_=xt_d[:, 0:NT])

            w1r = [cpool.tile([128, 1024], BF16, name=f"w1r{kc}",
                              tag=f"w1r{kc}") for kc in range(NCH)]
            w2r = [cpool.tile([128, 512], BF16, name=f"w2r{kc}",
                              tag=f"w2r{kc}") for kc in range(NCH)]
            w3r = [cpool.tile([128, 256], BF16, name=f"w3r{kc}",
                              tag=f"w3r{kc}") for kc in range(NCH)]
            # w1 in 128KB half-chunks (lo: out cols 0-511 feeding mc 0-3
            # of wave kc, hi: cols 512-1023), spread over all three DMA
            # queues in wave-consumption order so rt0's waves are PE-bound
            # from ~9us. The scalar queue's DMAs (its queue blocks on each
            # transfer) all land before its first ACT is needed (~23us);
            # sync and gpsimd carry the rest.
            def w1lo(kc, eng):
                eng.dma_start(out=w1r[kc][:, 0:512], in_=w1_d[kc][:, 0:512])

            def w1hi(kc, eng):
                eng.dma_start(out=w1r[kc][:, 512:1024],
                              in_=w1_d[kc][:, 512:1024])

            # lo halves stream on sync, hi halves on gpsimd, one pair per
            # wave; the scalar queue carries nothing (each queued DMA
            # blocks it for the transfer time and the first sin is needed
            # at ~12us). Small/late tensors follow the w1 stream.
            for kc in range(NCH):
                w1lo(kc, nc.sync)
            for kc in range(NCH):
                w1hi(kc, nc.gpsimd)
            w4t = cpool.tile([128, 10], F32, name="w4t", tag="w4t")
            nc.gpsimd.dma_start(out=w4t[:], in_=w4_d[:])
            bt = cpool.tile([128, 32], F32, name="bt", tag="bt")
            nc.gpsimd.dma_start(out=bt[:], in_=bias_d[:])
            b4t = cpool.tile([1, 1], F32, name="b4t", tag="b4t")
            nc.gpsimd.dma_start(out=b4t[:], in_=b4_d[:])
            onesr = cpool.tile([128, 128], BF16, name="onesr", tag="onesr")
            nc.gpsimd.dma_start(out=onesr[:], in_=ones_d[:])
            w4b = cpool.tile([128, 1024], BF16, name="w4b", tag="w4b")
            nc.gpsimd.dma_start(out=w4b[:], in_=w4b_d[:])
            # next row-tile's x, then the w2/w3 stream (all needed >20us)
            xr1 = xpool.tile([128, NT], BF16, name="xr", tag="xr")
            nc.vector.memset(xr1[:], 0.0)
            nc.sync.dma_start(out=xr1[0:11, :], in_=xt_d[:, NT:2 * NT])
            for kc in (0, 1, 2, 3):
                nc.sync.dma_start(out=w2r[kc][:], in_=w2_d[kc])
            for kc in (4, 5, 6, 7):
                nc.gpsimd.dma_start(out=w2r[kc][:], in_=w2_d[kc])
            for kc in (0, 1, 2, 3, 4, 5):
                nc.gpsimd.dma_start(out=w3r[kc][:], in_=w3_d[kc])
            for kc in (6, 7):
                nc.sync.dma_start(out=w3r[kc][:], in_=w3_d[kc])

            # PE warm-up: dep-free f32r matmuls bridge the first ~2us while
            # x/w0 DMA, keeping the HAM clock gate fed before real work.
            wmw = cpool.tile([128, 128], BF16, name="wmw", tag="wmw")
            nc.vector.memset(wmw[:], 0.0)
            wmx = cpool.tile([128, NT], BF16, name="wmx", tag="wmx")
            nc.vector.memset(wmx[:], 0.0)
            wmp = ppool.tile([128, NT], F32, name="wmp", tag="pt",
                             padded_shape=None)
            for i in range(2):
                nc.tensor.matmul(wmp[:, 0:NT], wmw[:], wmx[:],
                                 start=(i == 0), stop=(i == 1))

            PW = 2 * NT

            # ---- emitters ----------------------------------------------
            def mm_l0(mc, j):
                return dict(lhsT=w0r[:, 128 * mc:128 * (mc + 1)], rhs_idx=0)

            def emit_pair_l0(q, xr, bufs_):
                """L0 pair q: two dense K=128 bf16 matmuls (zero-padded
                weight rows) — same stream shape as every other layer."""
                return emit_pair(1, q, 1, mm_l0, [xr[:]], bufs_)

            def mm_l1(mc, j):
                kc = (mc + j) % NCH
                return dict(lhsT=w1r[kc][:, 128 * mc:128 * (mc + 1)],
                            rhs_idx=kc)

            def mm_l2(mc, j):
                b = mc // 4
                kcl = (mc + j) % 4
                return dict(lhsT=w2r[4 * b + kcl][:, (mc % 4) * 128:
                                                  (mc % 4) * 128 + 128],
                            rhs_idx=4 * b + kcl)

            def mm_l3(mc, j):
                bi = mc // 2
                kcl = (mc + j) % 2
                return dict(lhsT=w3r[2 * bi + kcl][:, (mc % 2) * 128:
                                                   (mc % 2) * 128 + 128],
                            rhs_idx=2 * bi + kcl)

            def emit_pair(lidx, q, nk, mm_args, hin, bufs_):
                """One 2-chunk group: both chunks share one 2-bank PSUM
                tile drained by a single wide Sin."""
                hp = hpool.tile([128, 2 * NT], BF16, name=f"h{lidx}_{q}",
                                tag=f"h{lidx}_{q}", bufs=bufs_)
                pt = ppool.tile([128, PW], F32, name="pt", tag="pt")
                for half in range(2):
                    mc = 2 * q + half
                    dst = pt[:, (half * NT):(half * NT) + NT]
                    for j in range(nk):
                        kw = mm_args(mc, j)
                        kc = kw.pop("rhs_idx")
                        nc.tensor.matmul(dst, rhs=hin[kc],
                                         start=(j == 0),
                                         stop=(j == nk - 1), **kw)
                nc.scalar.activation(hp[:], pt[:], SIN)
                return [hp[:, 0:NT], hp[:, NT:2 * NT]]

            def emit_l1_rt0(h1):
                """Row-tile 0's L1, K-chunk outer: 8 MMs per arriving W1
                chunk, accumulating into 4 simultaneously-held PSUM pairs.
                The PE consumes W1 in DMA arrival order."""
                pts = [ppool.tile([128, PW], F32, name="pt", tag="pt")
                       for _ in range(4)]
                for kc in range(NCH):
                    for mc in range(NCH):  # mc 0-3 need only the lo half
                        nc.tensor.matmul(
                            pts[mc // 2][:, (mc % 2) * NT:(mc % 2 + 1) * NT],
                            lhsT=w1r[kc][:, 128 * mc:128 * (mc + 1)],
                            rhs=h1[kc], start=(kc == 0), stop=(kc == NCH - 1))
                outs = []
                for q in range(4):
                    hp = hpool.tile([128, 2 * NT], BF16, name=f"h2_{q}",
                                    tag=f"h2_{q}", bufs=2)
                    nc.scalar.activation(hp[:], pts[q][:], SIN)
                    outs += [hp[:, 0:NT], hp[:, NT:2 * NT]]
                return outs

            def load_x(rt):
                cs = rt * NT
                xr = xpool.tile([128, NT], BF16, name="xr", tag="xr")
                nc.sync.dma_start(out=xr[0:11, :], in_=xt_d[:, cs:cs + NT])
                return xr

            def flush_tail(pend):
                # partition-reduce of the deferred row-tile's L4 accumulator
                # on the (otherwise idle) gpsimd engine, bias, and store
                p_rt, acc = pend
                red = xpool.tile([128, NT], F32, name="red", tag="red",
                                 bufs=2)
                nc.gpsimd.partition_all_reduce(red[:], acc[:], 128,
                                               bass_isa.ReduceOp.add)
                ot = xpool.tile([1, NT], F32, name="ot", tag="ot")
                nc.vector.tensor_scalar_add(ot[:], red[0:1, :], b4t[:])
                nc.sync.dma_start(out=o_d[p_rt:p_rt + 1, :], in_=ot[0:1, :])

            # ---- generic-biases fallback (v1 structure, unpacked L0) ---
            def emit_layer_generic(lidx, nk, mm_args, hin, bufs_):
                outs = []
                for mc in range(NCH):
                    pt = ppool.tile([128, PW], F32, name="pt", tag="pt")
                    dst = pt[:, 0:NT]
                    for j in range(nk):
                        kw = mm_args(mc, j)
                        kc = kw.pop("rhs_idx")
                        nc.tensor.matmul(dst, rhs=hin[kc],
                                         start=(j == 0),
                                         stop=(j == nk - 1), **kw)
                    h = hpool.tile([128, NT], BF16, name=f"h{lidx}_{mc}",
                                   tag=f"h{lidx}_{mc}", bufs=bufs_)
                    if lidx == 1:
                        nc.scalar.activation(h[:], dst, SIN)
                    else:
                        c = 8 * (lidx - 1) + mc
                        nc.scalar.activation(h[:], dst, SIN,
                                             bias=bt[:, c:c + 1])
                    outs.append(h[:])
                return outs

            def mm_l0_generic(mc, j):
                return dict(lhsT=w0r[:, 128 * mc:128 * (mc + 1)],
                            rhs_idx=mc)

            if not act_pairs:
                h1 = emit_layer_generic(1, 1, mm_l0_generic,
                                        [xr0[:]] * NCH, 2)
                h2 = emit_layer_generic(2, NCH, mm_l1, h1, 2)
                pend = None
                for rt in range(rt_count):
                    xrn = load_x(rt + 1) if rt + 1 < rt_count else None
                    h3 = emit_layer_generic(3, 4, mm_l2, h2, 1)
                    h1n = (emit_layer_generic(1, 1, mm_l0_generic,
                                              [xrn[:]] * NCH, 2)
                           if xrn is not None else [])
                    h4 = emit_layer_generic(4, 2, mm_l3, h3, 2)
                    h2n = (emit_layer_generic(2, NCH, mm_l1, h1n, 2)
                           if h1n else [])
                    h2 = h2n
                    if pend is not None:
                        flush_tail(pend)
                        pend = None
                    acf = xpool.tile([128, NT], F32, name="acf", tag="acf",
                                     bufs=2)
                    ach = xpool.tile([128, NT], F32, name="ach", tag="ach",
                                     bufs=2)
                    nc.vector.tensor_scalar_mul(acf[:], h4[0],
                                                w4t[:, 0:1])
                    nc.vector.tensor_scalar_mul(ach[:], h4[4],
                                                w4t[:, 4:5])
                    for kc in (1, 2, 3):
                        nc.vector.scalar_tensor_tensor(
                            acf[:], h4[kc], w4t[:, kc:kc + 1],
                            acf[:], mybir.AluOpType.mult,
                            mybir.AluOpType.add)
                    for kc in (5, 6, 7):
                        nc.vector.scalar_tensor_tensor(
                            ach[:], h4[kc], w4t[:, kc:kc + 1],
                            ach[:], mybir.AluOpType.mult,
                            mybir.AluOpType.add)
                    acc = xpool.tile([128, NT], BF16, name="acc", tag="acc",
                                     bufs=2)
                    nc.vector.tensor_tensor(acc[:], acf[:], ach[:],
                                            mybir.AluOpType.add)
                    pend = (rt, acc)
                flush_tail(pend)
            else:
                # ---- merged steady-state schedule ----------------------
                h1 = []
                for q in range(4):
                    h1 += emit_pair_l0(q, xr0, 2)
                h2 = emit_l1_rt0(h1)
                xrn = xr1
                pend = None
                for rt in range(rt_count):
                    last = rt == rt_count - 1
                    h3, h4, h1n = [None] * 8, [None] * 8, []
                    if last:
                        acf = ach = None
                    else:
                        acf = xpool.tile([128, NT], F32, name="acf",
                                         tag="acf", bufs=2)
                        ach = xpool.tile([128, NT], F32, name="ach",
                                         tag="ach", bufs=2)

                    def l1q(q):
                        h2[2 * q:2 * q + 2] = emit_pair(2, q, NCH, mm_l1,
                                                        h1, 2)

                    def l2q(q):
                        h3[2 * q:2 * q + 2] = emit_pair(3, q, 4, mm_l2,
                                                        h2, 1)

                    def l3q(q):
                        h4[2 * q:2 * q + 2] = emit_pair(4, q, 2, mm_l3,
                                                        h3, 2)
                        if last:
                            return  # L4 runs as a PE burst after l3q(3)
                        dstt = acf if q < 2 else ach
                        for mc in (2 * q, 2 * q + 1):
                            if mc % 4 == 0:
                                nc.vector.tensor_scalar_mul(
                                    dstt[:], h4[mc],
                                    w4t[:, mc:mc + 1])
                            else:
                                nc.vector.scalar_tensor_tensor(
                                    dstt[:], h4[mc],
                                    w4t[:, mc:mc + 1], dstt[:],
                                    mybir.AluOpType.mult,
                                    mybir.AluOpType.add)

                    def l0p(q):
                        if xrn is not None:
                            h1n.extend(emit_pair_l0(q, xrn, 2))

                    # L1(rt) rebuilds h2 in place: l2q(0)/l2q(1) read only
                    # chunks 0-3 (block 0), written by l1q(0)/l1q(1) above
                    # them; l2q(2)/l2q(3) read 4-7, written by l1q(2)/(3).
                    if rt > 0:
                        l1q(0)
                        l1q(1)
                        if pend is not None:
                            flush_tail(pend)
                            pend = None
                        l1q(2)
                    l2q(0)
                    l0p(0)
                    l2q(1)
                    l3q(0)
                    if rt > 0:
                        l1q(3)
                    l0p(1)
                    l3q(1)
                    l2q(2)
                    l2q(3)
                    l0p(2)
                    l3q(2)
                    l0p(3)
                    l3q(3)

                    if not last:
                        acc = xpool.tile([128, NT], F32, name="acc",
                                         tag="acc", bufs=2)
                        nc.vector.tensor_tensor(acc[:], acf[:], ach[:],
                                                mybir.AluOpType.add)
                        pend = (rt, acc)
                    h1 = h1n
                    xrn = load_x(rt + 2) if rt + 2 < rt_count else None

                # last row-tile's L4 as a short PE burst (accumulating
                # w4b partition-reduce matmuls) so no serial DVE chain is
                # exposed at the very end, then +b4 and store
                ptl = ppool.tile([128, PW], F32, name="ptl", tag="pt")
                for mc in range(NCH):
                    nc.tensor.matmul(ptl[:, 0:NT],
                                     lhsT=w4b[:, 128 * mc:128 * (mc + 1)],
                                     rhs=h4[mc],
                                     start=(mc == 0), stop=(mc == NCH - 1))
                ot = xpool.tile([1, NT], F32, name="ot", tag="ot")
                nc.vector.tensor_scalar_add(ot[:], ptl[0:1, 0:NT], b4t[:])
                nc.sync.dma_start(out=o_d[rt_count - 1:rt_count, :],
                                  in_=ot[0:1, :])

    nc.compile()
    return nc


def _get_program(act_pairs):
    key = act_pairs
    if key not in _PROGRAMS:
        _PROGRAMS[key] = _build_program(act_pairs=act_pairs)
    return _PROGRAMS[key]


def _rne11(x):
    """fp32 -> float32r grid: round-to-nearest-even keeping 11 mantissa bits
    (verified bit-identical to the on-chip f32r CAST)."""
    u = np.ascontiguousarray(x, np.float32).view(np.uint32).astype(np.uint64)
    bias = ((u >> 12) & 1) + (1 << 11) - 1
    return (((u + bias) >> 12) << 12).astype(np.uint32).view(np.float32)


def kernel(X, lb_X, ub_X, W0, b0, W1, b1, W2, b2, W3, b3, W4, b4):
    X = np.asarray(X, np.float32)
    lb = np.asarray(lb_X, np.float64)
    ub = np.asarray(ub_X, np.float64)
    W0 = np.asarray(W0, np.float64)
    b0 = np.asarray(b0, np.float64)

    # fold input normalization h = X*s + t into W0/b0:
    #   sin((X*s+t)@W0 + b0) = sin(X@(s[:,None]*W0) + (t@W0 + b0))
    # then expand K to 11 bf16 rows for full precision in bf16 matmuls:
    #   z = xh@W0h + xh@W0l + xl@W0h + bias_hi + bias_lo
    s = 2.0 / (ub - lb)
    t = -2.0 * lb / (ub - lb) - 1.0
    b0p = (b0 + t @ W0).astype(np.float32).reshape(1024)
    sW0 = (s[:, None] * W0).astype(np.float32)
    W0h = sW0.astype(_BF16NP)
    W0l = (sW0 - W0h.astype(np.float32)).astype(_BF16NP)
    bh = b0p.astype(_BF16NP)
    bl = (b0p - bh.astype(np.float32)).astype(_BF16NP)
    W0p = np.zeros((11, 1024), _BF16NP)
    W0p[0:3] = W0h
    W0p[3:6] = W0l
    W0p[6:9] = W0h
    W0p[9] = bh
    W0p[10] = bl

    W1 = np.asarray(W1, np.float32)
    W2 = np.asarray(W2, np.float32)
    W3 = np.asarray(W3, np.float32)
    W4 = np.asarray(W4, np.float32)
    b1 = np.asarray(b1, np.float32).reshape(1024)
    b2 = np.asarray(b2, np.float32).reshape(1024)
    b3 = np.asarray(b3, np.float32).reshape(1024)

    w1h = np.ascontiguousarray(W1.reshape(8, 128, 1024)).astype(_BF16NP)
    # W2: 2 blocks of 512x512 -> [4b+kcl] = W2[512b+128kcl:+128, 512b:+512]
    w2h = np.zeros((8, 128, 512), np.float32)
    for b in range(2):
        for kcl in range(4):
            w2h[4 * b + kcl] = W2[512 * b + 128 * kcl:512 * b + 128 * (kcl + 1),
                                  512 * b:512 * (b + 1)]
    # W3: 4 blocks of 256x256 -> [2bi+kcl] = W3[256bi+128kcl:+128, 256bi:+256]
    w3h = np.zeros((8, 128, 256), np.float32)
    for bi in range(4):
        for kcl in range(2):
            w3h[2 * bi + kcl] = W3[256 * bi + 128 * kcl:256 * bi + 128 * (kcl + 1),
                                   256 * bi:256 * (bi + 1)]
    # W4 [1024,1] -> [128,10]: col kc = W4[128kc:+128, 0]; cols 8-9 = ones
    # (stationary operand of the f32r partition-reduce matmul)
    w4h = np.ones((128, 10), np.float32)
    w4h[:, :8] = W4.reshape(8, 128).T
    # hidden-layer biases [128, 8] chunk-major columns (layers 1-3; layer 0's
    # bias is folded into the W0 ones-row)
    bh = np.zeros((128, 32), np.float32)
    for i, bb in enumerate([b1, b2, b3], start=1):
        bh[:, 8 * i:8 * (i + 1)] = bb.reshape(8, 128).T
    b4h = np.asarray(b4, np.float32).reshape(1, 1)

    w2h = w2h.astype(_BF16NP)
    w3h = w3h.astype(_BF16NP)
    # partition-reduce stationaries, zero-padded to full 128-wide weights
    # so the reduce matmuls keep the dense LDW/MM pipeline shape
    onesh = np.zeros((128, 128), _BF16NP)
    onesh[:, 0] = 1
    w4bp = np.zeros((128, 1024), np.float32)
    for mc in range(8):
        w4bp[:, 128 * mc] = w4h[:, mc]
    w4bp = w4bp.astype(_BF16NP)
    act_pairs = not (b1.any() or b2.any() or b3.any())
    nc = _get_program(act_pairs)

    in_maps = []
    for c in range(N_CORES):
        xt = np.ones((11, R), _BF16NP)  # rows 9-10 = ones (bias rows)
        xc = X[c * R:(c + 1) * R].T
        xch = xc.astype(_BF16NP)
        xt[0:3] = xch
        xt[3:6] = xch
        xt[6:9] = (xc - xch.astype(np.float32)).astype(_BF16NP)
        in_maps.append({
            "xt": xt, "w0": W0p, "w1": w1h, "w2": w2h, "w3": w3h,
            "w4": w4h, "bias": bh, "b4": b4h,
            "onesr": onesh, "w4b": w4bp,
        })

    trace = bool(int(os.environ.get("KERNEL_TRACE", "0")))
    res = run_bass_kernel_spmd(nc, in_maps, list(range(N_CORES)), trace=trace)
    global LAST_RESULTS
    LAST_RESULTS = res

    out = np.concatenate([res.results[c]["o"].reshape(R) for c in range(N_CORES)])
    return out.reshape(N_FULL, 1).astype(np.float32)


# revision 17
# speedup vs baseline: 1.0184x; 1.0022x over previous
"""BsPINN forward MLP on 8 TRN2 NeuronCores (Bass/Tile), data-parallel over rows.

Network (per reference):
  h = 2*(X-lb)/(ub-lb)-1          [N,3]   (folded into W0/b0 on host)
  h = sin(h @ W0 + b0)            [N,1024]
  h = sin(h @ W1 + b1)            [N,1024] dense
  h = sin(h @ (W2*m2) + b2)       [N,1024] block-diag 2x(512x512)
  h = sin(h @ (W3*m3) + b3)       [N,1024] block-diag 4x(256x256)
  out = h @ W4 + b4               [N,1]

Design notes (v2; v1 measured 960us, see kernel_baseline.py):
  * Activations kept feature-major on chip (hT: features->partitions,
    rows->free); out_chunkT = W_chunk.T @ hT via nc.tensor.matmul, moving
    free dim 512 (one PSUM bank). Matmuls in float32r (fp32 RNE-rounded to
    11 mantissa bits on host), streaming 1 cycle/row.
  * Block-diagonal masks exploited by multiplying only in-block K-chunks
    (L2: 4 of 8, L3: 2 of 8) -- 60.3 GFLOP/core instead of 103.
  * L0 (K=4) thin matmuls are packed 2-per-PSUM-pair into DIFFERENT 32-row
    groups (tile_position (0,0) and (32,0), with x and W0 replicated at
    partition 32): the row-group LDWEIGHTS of the second MM overlaps the
    first MM's stream and the two MMs run concurrently, ~330ns per pair
    instead of 2x320ns serialized (v1 trace: thin LDW cannot hide behind a
    full-array MM).
  * Single merged per-row-tile schedule instead of v1's two phases, using
    the block-diag locality (L3 pair q needs only L2 pair q's output):
    [L1q0 L1q1 L1q2 L2q0 L0'p0 L2q1 L1q3 L0'p1 L2q2 L3q0 L0'p2 L2q3 L0'p3
     L3q1 L3q2 L3q3] keeps the ACT(sin) queue's duty even (~68%) so PSUM
    pair rotation never blocks the PE (v1's phase A was ACT-saturated).
  * Row-tile 0's L1 runs K-chunk-outer "waves" (8 MMs per arriving W1
    chunk, accumulating into 4 held PSUM pairs) so the PE consumes W1
    strictly in DMA arrival order; W1 chunks are spread over the scalar/
    sync HWDGE queues + the gpsimd SWDGE queue. Steady state starts ~20us
    vs ~34us in v1.
  * Sin on the scalar engine drains two PSUM banks per instruction; zero
    hidden biases (true here) let the bias ride W0's ones-row; a
    bias-general fallback program is compiled if biases are nonzero.
  * L4 (1024->1) runs as DVE per-partition multiply-accumulate plus a
    single f32r ones-matmul partition reduce, deferred one row-tile.
"""
import os
import numpy as np
import ml_dtypes

_BF16NP = ml_dtypes.bfloat16

try:  # run_bass_kernel_spmd(trace=True) imports this; absent in some images
    from antenv import axon_hooks as _axon_hooks  # noqa: F401
except ImportError:
    import sys
    import types
    _m = types.ModuleType("antenv.axon_hooks")
    _hook = [None]
    _m.set_axon_ntff_profile_hook = lambda h: _hook.__setitem__(0, h)
    _m.get_axon_ntff_profile_hook = lambda: _hook[0]
    sys.modules["antenv.axon_hooks"] = _m

import concourse.bass as bass
import concourse.tile as tile
from concourse import bacc, mybir
from concourse.bass_utils import run_bass_kernel_spmd
from concourse import bass_isa

N_CORES = 8
N_FULL = 131072
R = N_FULL // N_CORES          # 16384 rows per core
NT = 512                       # matmul moving free dim (one PSUM bank, fp32)
RT = R // NT                   # 32 row tiles per core
NCH = 8                        # feature chunks (1024 / 128)

F32 = mybir.dt.float32
F32R = mybir.dt.float32r
BF16 = mybir.dt.bfloat16
SIN = mybir.ActivationFunctionType.Sin
IDENT = mybir.ActivationFunctionType.Identity

LAST_RESULTS = None
_PROGRAMS = {}


def _build_program(rt_count=RT, n_cores=N_CORES, act_pairs=True):
    nc = bacc.Bacc("TRN2", target_bir_lowering=False, debug=False,
                   num_devices=n_cores)

    xt_d = nc.dram_tensor("xt", [11, R], BF16, kind="ExternalInput").ap()
    w0_d = nc.dram_tensor("w0", [11, 1024], BF16, kind="ExternalInput").ap()
    w1_d = nc.dram_tensor("w1", [8, 128, 1024], BF16, kind="ExternalInput").ap()
    w2_d = nc.dram_tensor("w2", [8, 128, 512], BF16, kind="ExternalInput").ap()
    w3_d = nc.dram_tensor("w3", [8, 128, 256], BF16, kind="ExternalInput").ap()
    w4_d = nc.dram_tensor("w4", [128, 10], F32, kind="ExternalInput").ap()
    bias_d = nc.dram_tensor("bias", [128, 32], F32, kind="ExternalInput").ap()
    b4_d = nc.dram_tensor("b4", [1, 1], F32, kind="ExternalInput").ap()
    ones_d = nc.dram_tensor("onesr", [128, 128], BF16, kind="ExternalInput").ap()
    w4b_d = nc.dram_tensor("w4b", [128, 1024], BF16, kind="ExternalInput").ap()
    o_d = nc.dram_tensor("o", [RT, NT], F32, kind="ExternalOutput").ap()

    with tile.TileContext(nc) as tc:
        with (
            tc.tile_pool(name="const", bufs=1) as cpool,
            tc.tile_pool(name="hbuf", bufs=2) as hpool,
            tc.tile_pool(name="xio", bufs=2) as xpool,
            tc.tile_pool(name="psum", bufs=4, space="PSUM") as ppool,
        ):
            # ---- one-time weight/bias setup ----------------------------
            # Weights arrive host-pre-rounded to the f32r grid and DMA
            # straight into f32r tiles. W0 (with its replica at partition
            # 32 for the row-group-packed L0) leads the scalar queue so
            # L0(rt0) can run at ~1us; W1 chunks are consumed in strict
            # arrival order by rt0's K-outer waves, spread over scalar/
            # sync/gpsimd so all 8 land by ~11us.
            # W0 row 3 holds b0 (the rhs carries a matching ones-row).
            # L0 runs as plain dense K=128 bf16 matmuls: weight rows 11-127
            # are zero, and the x tiles' rows 11-127 are zeroed once at
            # startup (the 2 rotation buffers keep their zeros; each
            # row-tile's DMA only rewrites rows 0-10). Zero weights x
            # finite garbage would be fine, but zero x zero avoids any
            # NaN-pattern risk.
            w0r = cpool.tile([128, 1024], BF16, name="w0r", tag="w0r")
            nc.vector.memset(w0r[:], 0.0)
            nc.scalar.dma_start(out=w0r[0:11, :], in_=w0_d[:])

            xr0 = xpool.tile([128, NT], BF16, name="xr", tag="xr")
            nc.vector.memset(xr0[:], 0.0)
            nc.sync.dma_start(out=xr0[0:11, :], in_=xt_d[:, 0:NT])

            w1r = [cpool.tile([128, 1024], BF16, name=f"w1r{kc}",
                              tag=f"w1r{kc}") for kc in range(NCH)]
            w2r = [cpool.tile([128, 512], BF16, name=f"w2r{kc}",
                              tag=f"w2r{kc}") for kc in range(NCH)]
            w3r = [cpool.tile([128, 256], BF16, name=f"w3r{kc}",
                              tag=f"w3r{kc}") for kc in range(NCH)]
            # w1 in 128KB half-chunks (lo: out cols 0-511 feeding mc 0-3
            # of wave kc, hi: cols 512-1023), spread over all three DMA
            # queues in wave-consumption order so rt0's waves are PE-bound
            # from ~9us. The scalar queue's DMAs (its queue blocks on each
            # transfer) all land before its first ACT is needed (~23us);
            # sync and gpsimd carry the rest.
            def w1lo(kc, eng):
                eng.dma_start(out=w1r[kc][:, 0:512], in_=w1_d[kc][:, 0:512])

            def w1hi(kc, eng):
                eng.dma_start(out=w1r[kc][:, 512:1024],
                              in_=w1_d[kc][:, 512:1024])

            # lo halves stream on sync, hi halves on gpsimd, one pair per
            # wave; the scalar queue carries nothing (each queued DMA
            # blocks it for the transfer time and the first sin is needed
            # at ~12us). Small/late tensors follow the w1 stream.
            for kc in range(NCH):
                w1lo(kc, nc.sync)
            for kc in range(NCH):
                w1hi(kc, nc.gpsimd)
            w4t = cpool.tile([128, 10], F32, name="w4t", tag="w4t")
            nc.gpsimd.dma_start(out=w4t[:], in_=w4_d[:])
            bt = cpool.tile([128, 32], F32, name="bt", tag="bt")
            nc.gpsimd.dma_start(out=bt[:], in_=bias_d[:])
            b4t = cpool.tile([1, 1], F32, name="b4t", tag="b4t")
            nc.gpsimd.dma_start(out=b4t[:], in_=b4_d[:])
            onesr = cpool.tile([128, 128], BF16, name="onesr", tag="onesr")
            nc.gpsimd.dma_start(out=onesr[:], in_=ones_d[:])
            w4b = cpool.tile([128, 1024], BF16, name="w4b", tag="w4b")
            nc.gpsimd.dma_start(out=w4b[:], in_=w4b_d[:])
            # next row-tile's x, then the w2/w3 stream (all needed >20us)
            xr1 = xpool.tile([128, NT], BF16, name="xr", tag="xr")
            nc.vector.memset(xr1[:], 0.0)
            nc.sync.dma_start(out=xr1[0:11, :], in_=xt_d[:, NT:2 * NT])
            for kc in (0, 1, 2, 3):
                nc.sync.dma_start(out=w2r[kc][:], in_=w2_d[kc])
            for kc in (4, 5, 6, 7):
                nc.gpsimd.dma_start(out=w2r[kc][:], in_=w2_d[kc])
            for kc in (0, 1, 2, 3, 4, 5):
                nc.gpsimd.dma_start(out=w3r[kc][:], in_=w3_d[kc])
            for kc in (6, 7):
                nc.sync.dma_start(out=w3r[kc][:], in_=w3_d[kc])

            PW = 2 * NT

            # ---- emitters ----------------------------------------------
            def mm_l0(mc, j):
                return dict(lhsT=w0r[:, 128 * mc:128 * (mc + 1)], rhs_idx=0)

            def emit_pair_l0(q, xr, bufs_):
                """L0 pair q: two dense K=128 bf16 matmuls (zero-padded
                weight rows) — same stream shape as every other layer."""
                return emit_pair(1, q, 1, mm_l0, [xr[:]], bufs_)

            def mm_l1(mc, j):
                kc = (mc + j) % NCH
                return dict(lhsT=w1r[kc][:, 128 * mc:128 * (mc + 1)],
                            rhs_idx=kc)

            def mm_l2(mc, j):
                b = mc // 4
                kcl = (mc + j) % 4
                return dict(lhsT=w2r[4 * b + kcl][:, (mc % 4) * 128:
                                                  (mc % 4) * 128 + 128],
                            rhs_idx=4 * b + kcl)

            def mm_l3(mc, j):
                bi = mc // 2
                kcl = (mc + j) % 2
                return dict(lhsT=w3r[2 * bi + kcl][:, (mc % 2) * 128:
                                                   (mc % 2) * 128 + 128],
                            rhs_idx=2 * bi + kcl)

            def emit_pair(lidx, q, nk, mm_args, hin, bufs_):
                """One 2-chunk group: both chunks share one 2-bank PSUM
                tile drained by a single wide Sin."""
                hp = hpool.tile([128, 2 * NT], BF16, name=f"h{lidx}_{q}",
                                tag=f"h{lidx}_{q}", bufs=bufs_)
                pt = ppool.tile([128, PW], F32, name="pt", tag="pt")
                for half in range(2):
                    mc = 2 * q + half
                    dst = pt[:, (half * NT):(half * NT) + NT]
                    for j in range(nk):
                        kw = mm_args(mc, j)
                        kc = kw.pop("rhs_idx")
                        nc.tensor.matmul(dst, rhs=hin[kc],
                                         start=(j == 0),
                                         stop=(j == nk - 1), **kw)
                nc.scalar.activation(hp[:], pt[:], SIN)
                return [hp[:, 0:NT], hp[:, NT:2 * NT]]

            def emit_l1_rt0(h1):
                """Row-tile 0's L1, K-chunk outer: 8 MMs per arriving W1
                chunk, accumulating into 4 simultaneously-held PSUM pairs.
                The PE consumes W1 in DMA arrival order."""
                pts = [ppool.tile([128, PW], F32, name="pt", tag="pt")
                       for _ in range(4)]
                for kc in range(NCH):
                    for mc in range(NCH):  # mc 0-3 need only the lo half
                        nc.tensor.matmul(
                            pts[mc // 2][:, (mc % 2) * NT:(mc % 2 + 1) * NT],
                            lhsT=w1r[kc][:, 128 * mc:128 * (mc + 1)],
                            rhs=h1[kc], start=(kc == 0), stop=(kc == NCH - 1))
                outs = []
                for q in range(4):
                    hp = hpool.tile([128, 2 * NT], BF16, name=f"h2_{q}",
                                    tag=f"h2_{q}", bufs=2)
                    nc.scalar.activation(hp[:], pts[q][:], SIN)
                    outs += [hp[:, 0:NT], hp[:, NT:2 * NT]]
                return outs

            def load_x(rt):
                cs = rt * NT
                xr = xpool.tile([128, NT], BF16, name="xr", tag="xr")
                nc.sync.dma_start(out=xr[0:11, :], in_=xt_d[:, cs:cs + NT])
                return xr

            def flush_tail(pend):
                # partition-reduce of the deferred row-tile's L4 accumulator
                # on the (otherwise idle) gpsimd engine, bias, and store
                p_rt, acc = pend
                red = xpool.tile([128, NT], F32, name="red", tag="red",
                                 bufs=2)
                nc.gpsimd.partition_all_reduce(red[:], acc[:], 128,
                                               bass_isa.ReduceOp.add)
                ot = xpool.tile([1, NT], F32, name="ot", tag="ot")
                nc.vector.tensor_scalar_add(ot[:], red[0:1, :], b4t[:])
                nc.sync.dma_start(out=o_d[p_rt:p_rt + 1, :], in_=ot[0:1, :])

            # ---- generic-biases fallback (v1 structure, unpacked L0) ---
            def emit_layer_generic(lidx, nk, mm_args, hin, bufs_):
                outs = []
                for mc in range(NCH):
                    pt = ppool.tile([128, PW], F32, name="pt", tag="pt")
                    dst = pt[:, 0:NT]
                    for j in range(nk):
                        kw = mm_args(mc, j)
                        kc = kw.pop("rhs_idx")
                        nc.tensor.matmul(dst, rhs=hin[kc],
                                         start=(j == 0),
                                         stop=(j == nk - 1), **kw)
                    h = hpool.tile([128, NT], BF16, name=f"h{lidx}_{mc}",
                                   tag=f"h{lidx}_{mc}", bufs=bufs_)
                    if lidx == 1:
                        nc.scalar.activation(h[:], dst, SIN)
                    else:
                        c = 8 * (lidx - 1) + mc
                        nc.scalar.activation(h[:], dst, SIN,
                                             bias=bt[:, c:c + 1])
                    outs.append(h[:])
                return outs

            def mm_l0_generic(mc, j):
                return dict(lhsT=w0r[:, 128 * mc:128 * (mc + 1)],
                            rhs_idx=mc)

            if not act_pairs:
                h1 = emit_layer_generic(1, 1, mm_l0_generic,
                                        [xr0[:]] * NCH, 2)
                h2 = emit_layer_generic(2, NCH, mm_l1, h1, 2)
                pend = None
                for rt in range(rt_count):
                    xrn = load_x(rt + 1) if rt + 1 < rt_count else None
                    h3 = emit_layer_generic(3, 4, mm_l2, h2, 1)
                    h1n = (emit_layer_generic(1, 1, mm_l0_generic,
                                              [xrn[:]] * NCH, 2)
                           if xrn is not None else [])
                    h4 = emit_layer_generic(4, 2, mm_l3, h3, 2)
                    h2n = (emit_layer_generic(2, NCH, mm_l1, h1n, 2)
                           if h1n else [])
                    h2 = h2n
                    if pend is not None:
                        flush_tail(pend)
                        pend = None
                    acf = xpool.tile([128, NT], F32, name="acf", tag="acf",
                                     bufs=2)
                    ach = xpool.tile([128, NT], F32, name="ach", tag="ach",
                                     bufs=2)
                    nc.vector.tensor_scalar_mul(acf[:], h4[0],
                                                w4t[:, 0:1])
                    nc.vector.tensor_scalar_mul(ach[:], h4[4],
                                                w4t[:, 4:5])
                    for kc in (1, 2, 3):
                        nc.vector.scalar_tensor_tensor(
                            acf[:], h4[kc], w4t[:, kc:kc + 1],
                            acf[:], mybir.AluOpType.mult,
                            mybir.AluOpType.add)
                    for kc in (5, 6, 7):
                        nc.vector.scalar_tensor_tensor(
                            ach[:], h4[kc], w4t[:, kc:kc + 1],
                            ach[:], mybir.AluOpType.mult,
                            mybir.AluOpType.add)
                    acc = xpool.tile([128, NT], BF16, name="acc", tag="acc",
                                     bufs=2)
                    nc.vector.tensor_tensor(acc[:], acf[:], ach[:],
                                            mybir.AluOpType.add)
                    pend = (rt, acc)
                flush_tail(pend)
            else:
                # ---- merged steady-state schedule ----------------------
                h1 = []
                for q in range(4):
                    h1 += emit_pair_l0(q, xr0, 2)
                h2 = emit_l1_rt0(h1)
                xrn = xr1
                pend = None
                for rt in range(rt_count):
                    last = rt == rt_count - 1
                    h3, h4, h1n = [None] * 8, [None] * 8, []
                    if last:
                        acf = ach = None
                    else:
                        acf = xpool.tile([128, NT], F32, name="acf",
                                         tag="acf", bufs=2)
                        ach = xpool.tile([128, NT], F32, name="ach",
                                         tag="ach", bufs=2)

                    def l1q(q):
                        h2[2 * q:2 * q + 2] = emit_pair(2, q, NCH, mm_l1,
                                                        h1, 2)

                    def l2q(q):
                        h3[2 * q:2 * q + 2] = emit_pair(3, q, 4, mm_l2,
                                                        h2, 1)

                    def l3q(q):
                        h4[2 * q:2 * q + 2] = emit_pair(4, q, 2, mm_l3,
                                                        h3, 2)
                        if last:
                            return  # L4 runs as a PE burst after l3q(3)
                        dstt = acf if q < 2 else ach
                        for mc in (2 * q, 2 * q + 1):
                            if mc % 4 == 0:
                                nc.vector.tensor_scalar_mul(
                                    dstt[:], h4[mc],
                                    w4t[:, mc:mc + 1])
                            else:
                                nc.vector.scalar_tensor_tensor(
                                    dstt[:], h4[mc],
                                    w4t[:, mc:mc + 1], dstt[:],
                                    mybir.AluOpType.mult,
                                    mybir.AluOpType.add)

                    def l0p(q):
                        if xrn is not None:
                            h1n.extend(emit_pair_l0(q, xrn, 2))

                    # L1(rt) rebuilds h2 in place: l2q(0)/l2q(1) read only
                    # chunks 0-3 (block 0), written by l1q(0)/l1q(1) above
                    # them; l2q(2)/l2q(3) read 4-7, written by l1q(2)/(3).
                    if rt > 0:
                        l1q(0)
                        l1q(1)
                        if pend is not None:
                            flush_tail(pend)
                            pend = None
                        l1q(2)
                    l2q(0)
                    l0p(0)
                    l2q(1)
                    l3q(0)
                    if rt > 0:
                        l1q(3)
                    l0p(1)
                    l3q(1)
                    l2q(2)
                    l2q(3)
                    l0p(2)
                    l3q(2)
                    l0p(3)
                    l3q(3)

                    if not last:
                        acc = xpool.tile([128, NT], F32, name="acc",
                                         tag="acc", bufs=2)
                        nc.vector.tensor_tensor(acc[:], acf[:], ach[:],
                                                mybir.AluOpType.add)
                        pend = (rt, acc)
                    h1 = h1n
                    xrn = load_x(rt + 2) if rt + 2 < rt_count else None

                # last row-tile's L4 as a short PE burst (accumulating
                # w4b partition-reduce matmuls) so no serial DVE chain is
                # exposed at the very end, then +b4 and store
                ptl = ppool.tile([128, PW], F32, name="ptl", tag="pt")
                for mc in range(NCH):
                    nc.tensor.matmul(ptl[:, 0:NT],
                                     lhsT=w4b[:, 128 * mc:128 * (mc + 1)],
                                     rhs=h4[mc],
                                     start=(mc == 0), stop=(mc == NCH - 1))
                ot = xpool.tile([1, NT], F32, name="ot", tag="ot")
                nc.vector.tensor_scalar_add(ot[:], ptl[0:1, 0:NT], b4t[:])
                nc.sync.dma_start(out=o_d[rt_count - 1:rt_count, :],
                                  in_=ot[0:1, :])

    nc.compile()
    return nc


def _get_program(act_pairs):
    key = act_pairs
    if key not in _PROGRAMS:
        _PROGRAMS[key] = _build_program(act_pairs=act_pairs)
    return _PROGRAMS[key]


def _rne11(x):
    """fp32 -> float32r grid: round-to-nearest-even keeping 11 mantissa bits
    (verified bit-identical to the on-chip f32r CAST)."""
    u = np.ascontiguousarray(x, np.float32).view(np.uint32).astype(np.uint64)
    bias = ((u >> 12) & 1) + (1 << 11) - 1
    return (((u + bias) >> 12) << 12).astype(np.uint32).view(np.float32)


def kernel(X, lb_X, ub_X, W0, b0, W1, b1, W2, b2, W3, b3, W4, b4):
    X = np.asarray(X, np.float32)
    lb = np.asarray(lb_X, np.float64)
    ub = np.asarray(ub_X, np.float64)
    W0 = np.asarray(W0, np.float64)
    b0 = np.asarray(b0, np.float64)

    # fold input normalization h = X*s + t into W0/b0:
    #   sin((X*s+t)@W0 + b0) = sin(X@(s[:,None]*W0) + (t@W0 + b0))
    # then expand K to 11 bf16 rows for full precision in bf16 matmuls:
    #   z = xh@W0h + xh@W0l + xl@W0h + bias_hi + bias_lo
    s = 2.0 / (ub - lb)
    t = -2.0 * lb / (ub - lb) - 1.0
    b0p = (b0 + t @ W0).astype(np.float32).reshape(1024)
    sW0 = (s[:, None] * W0).astype(np.float32)
    W0h = sW0.astype(_BF16NP)
    W0l = (sW0 - W0h.astype(np.float32)).astype(_BF16NP)
    bh = b0p.astype(_BF16NP)
    bl = (b0p - bh.astype(np.float32)).astype(_BF16NP)
    W0p = np.zeros((11, 1024), _BF16NP)
    W0p[0:3] = W0h
    W0p[3:6] = W0l
    W0p[6:9] = W0h
    W0p[9] = bh
    W0p[10] = bl

    W1 = np.asarray(W1, np.float32)
    W2 = np.asarray(W2, np.float32)
    W3 = np.asarray(W3, np.float32)
    W4 = np.asarray(W4, np.float32)
    b1 = np.asarray(b1, np.float32).reshape(1024)
    b2 = np.asarray(b2, np.float32).reshape(1024)
    b3 = np.asarray(b3, np.float32).reshape(1024)

    w1h = np.ascontiguousarray(W1.reshape(8, 128, 1024)).astype(_BF16NP)
    # W2: 2 blocks of 512x512 -> [4b+kcl] = W2[512b+128kcl:+128, 512b:+512]
    w2h = np.zeros((8, 128, 512), np.float32)
    for b in range(2):
        for kcl in range(4):
            w2h[4 * b + kcl] = W2[512 * b + 128 * kcl:512 * b + 128 * (kcl + 1),
                                  512 * b:512 * (b + 1)]
    # W3: 4 blocks of 256x256 -> [2bi+kcl] = W3[256bi+128kcl:+128, 256bi:+256]
    w3h = np.zeros((8, 128, 256), np.float32)
    for bi in range(4):
        for kcl in range(2):
            w3h[2 * bi + kcl] = W3[256 * bi + 128 * kcl:256 * bi + 128 * (kcl + 1),
                                   256 * bi:256 * (bi + 1)]
    # W4 [1024,1] -> [128,10]: col kc = W4[128kc:+128, 0]; cols 8-9 = ones
    # (stationary operand of the f32r partition-reduce matmul)
    w4h = np.ones((128, 10), np.float32)
    w4h[:, :8] = W4.reshape(8, 128).T
    # hidden-layer biases [128, 8] chunk-major columns (layers 1-3; layer 0's
    # bias is folded into the W0 ones-row)
    bh = np.zeros((128, 32), np.float32)
    for i, bb in enumerate([b1, b2, b3], start=1):
        bh[:, 8 * i:8 * (i + 1)] = bb.reshape(8, 128).T
    b4h = np.asarray(b4, np.float32).reshape(1, 1)

    w2h = w2h.astype(_BF16NP)
    w3h = w3h.astype(_BF16NP)
    # partition-reduce stationaries, zero-padded to full 128-wide weights
    # so the reduce matmuls keep the dense LDW/MM pipeline shape
    onesh = np.zeros((128, 128), _BF16NP)
    onesh[:, 0] = 1
    w4bp = np.zeros((128, 1024), np.float32)
    for mc in range(8):
        w4bp[:, 128 * mc] = w4h[:, mc]
    w4bp = w4bp.astype(_BF16NP)
    act_pairs = not (b1.any() or b2.any() or b3.any())
    nc = _get_program(act_pairs)

    in_maps = []
    for c in range(N_CORES):
        xt = np.ones((11, R), _BF16NP)  # rows 9-10 = ones (bias rows)
        xc = X[c * R:(c + 1) * R].T
        xch = xc.astype(_BF16NP)
        xt[0:3] = xch
        xt[3:6] = xch
        xt[6:9] = (xc - xch.astype(np.float32)).astype(_BF16NP)
        in_maps.append({
            "xt": xt, "w0": W0p, "w1": w1h, "w2": w2h, "w3": w3h,
            "w4": w4h, "bias": bh, "b4": b4h,
            "onesr": onesh, "w4b": w4bp,
        })

    trace = bool(int(os.environ.get("KERNEL_TRACE", "0")))
    res = run_bass_kernel_spmd(nc, in_maps, list(range(N_CORES)), trace=trace)
    global LAST_RESULTS
    LAST_RESULTS = res

    out = np.concatenate([res.results[c]["o"].reshape(R) for c in range(N_CORES)])
    return out.reshape(N_FULL, 1).astype(np.float32)


# revision 18
# speedup vs baseline: 1.0231x; 1.0046x over previous
"""BsPINN forward MLP on 8 TRN2 NeuronCores (Bass/Tile), data-parallel over rows.

Network (per reference):
  h = 2*(X-lb)/(ub-lb)-1          [N,3]   (folded into W0/b0 on host)
  h = sin(h @ W0 + b0)            [N,1024]
  h = sin(h @ W1 + b1)            [N,1024] dense
  h = sin(h @ (W2*m2) + b2)       [N,1024] block-diag 2x(512x512)
  h = sin(h @ (W3*m3) + b3)       [N,1024] block-diag 4x(256x256)
  out = h @ W4 + b4               [N,1]

Design notes (v2; v1 measured 960us, see kernel_baseline.py):
  * Activations kept feature-major on chip (hT: features->partitions,
    rows->free); out_chunkT = W_chunk.T @ hT via nc.tensor.matmul, moving
    free dim 512 (one PSUM bank). Matmuls in float32r (fp32 RNE-rounded to
    11 mantissa bits on host), streaming 1 cycle/row.
  * Block-diagonal masks exploited by multiplying only in-block K-chunks
    (L2: 4 of 8, L3: 2 of 8) -- 60.3 GFLOP/core instead of 103.
  * L0 (K=4) thin matmuls are packed 2-per-PSUM-pair into DIFFERENT 32-row
    groups (tile_position (0,0) and (32,0), with x and W0 replicated at
    partition 32): the row-group LDWEIGHTS of the second MM overlaps the
    first MM's stream and the two MMs run concurrently, ~330ns per pair
    instead of 2x320ns serialized (v1 trace: thin LDW cannot hide behind a
    full-array MM).
  * Single merged per-row-tile schedule instead of v1's two phases, using
    the block-diag locality (L3 pair q needs only L2 pair q's output):
    [L1q0 L1q1 L1q2 L2q0 L0'p0 L2q1 L1q3 L0'p1 L2q2 L3q0 L0'p2 L2q3 L0'p3
     L3q1 L3q2 L3q3] keeps the ACT(sin) queue's duty even (~68%) so PSUM
    pair rotation never blocks the PE (v1's phase A was ACT-saturated).
  * Row-tile 0's L1 runs K-chunk-outer "waves" (8 MMs per arriving W1
    chunk, accumulating into 4 held PSUM pairs) so the PE consumes W1
    strictly in DMA arrival order; W1 chunks are spread over the scalar/
    sync HWDGE queues + the gpsimd SWDGE queue. Steady state starts ~20us
    vs ~34us in v1.
  * Sin on the scalar engine drains two PSUM banks per instruction; zero
    hidden biases (true here) let the bias ride W0's ones-row; a
    bias-general fallback program is compiled if biases are nonzero.
  * L4 (1024->1) runs as DVE per-partition multiply-accumulate plus a
    single f32r ones-matmul partition reduce, deferred one row-tile.
"""
import os
import numpy as np
import ml_dtypes

_BF16NP = ml_dtypes.bfloat16

try:  # run_bass_kernel_spmd(trace=True) imports this; absent in some images
    from antenv import axon_hooks as _axon_hooks  # noqa: F401
except ImportError:
    import sys
    import types
    _m = types.ModuleType("antenv.axon_hooks")
    _hook = [None]
    _m.set_axon_ntff_profile_hook = lambda h: _hook.__setitem__(0, h)
    _m.get_axon_ntff_profile_hook = lambda: _hook[0]
    sys.modules["antenv.axon_hooks"] = _m

import concourse.bass as bass
import concourse.tile as tile
from concourse import bacc, mybir
from concourse.bass_utils import run_bass_kernel_spmd
from concourse import bass_isa

N_CORES = 8
N_FULL = 131072
R = N_FULL // N_CORES          # 16384 rows per core
NT = 512                       # matmul moving free dim (one PSUM bank, fp32)
RT = R // NT                   # 32 row tiles per core
NCH = 8                        # feature chunks (1024 / 128)

F32 = mybir.dt.float32
F32R = mybir.dt.float32r
BF16 = mybir.dt.bfloat16
SIN = mybir.ActivationFunctionType.Sin
IDENT = mybir.ActivationFunctionType.Identity

LAST_RESULTS = None
_PROGRAMS = {}


def _build_program(rt_count=RT, n_cores=N_CORES, act_pairs=True):
    nc = bacc.Bacc("TRN2", target_bir_lowering=False, debug=False,
                   num_devices=n_cores)

    xt_d = nc.dram_tensor("xt", [11, R], BF16, kind="ExternalInput").ap()
    w0_d = nc.dram_tensor("w0", [128, 1024], BF16, kind="ExternalInput").ap()
    x0p_d = nc.dram_tensor("x0p", [128, 2 * NT], BF16, kind="ExternalInput").ap()
    w1_d = nc.dram_tensor("w1", [8, 128, 1024], BF16, kind="ExternalInput").ap()
    w2_d = nc.dram_tensor("w2", [8, 128, 512], BF16, kind="ExternalInput").ap()
    w3_d = nc.dram_tensor("w3", [8, 128, 256], BF16, kind="ExternalInput").ap()
    w4_d = nc.dram_tensor("w4", [128, 10], F32, kind="ExternalInput").ap()
    bias_d = nc.dram_tensor("bias", [128, 32], F32, kind="ExternalInput").ap()
    b4_d = nc.dram_tensor("b4", [1, 1], F32, kind="ExternalInput").ap()
    ones_d = nc.dram_tensor("onesr", [128, 128], BF16, kind="ExternalInput").ap()
    w4b_d = nc.dram_tensor("w4b", [128, 1024], BF16, kind="ExternalInput").ap()
    o_d = nc.dram_tensor("o", [RT, NT], F32, kind="ExternalOutput").ap()

    with tile.TileContext(nc) as tc:
        with (
            tc.tile_pool(name="const", bufs=1) as cpool,
            tc.tile_pool(name="hbuf", bufs=2) as hpool,
            tc.tile_pool(name="xio", bufs=2) as xpool,
            tc.tile_pool(name="psum", bufs=4, space="PSUM") as ppool,
        ):
            # ---- one-time weight/bias setup ----------------------------
            # Weights arrive host-pre-rounded to the f32r grid and DMA
            # straight into f32r tiles. W0 (with its replica at partition
            # 32 for the row-group-packed L0) leads the scalar queue so
            # L0(rt0) can run at ~1us; W1 chunks are consumed in strict
            # arrival order by rt0's K-outer waves, spread over scalar/
            # sync/gpsimd so all 8 land by ~11us.
            # W0 row 3 holds b0 (the rhs carries a matching ones-row).
            # L0 runs as plain dense K=128 bf16 matmuls: weight rows 11-127
            # are zero, and the x tiles' rows 11-127 are zeroed once at
            # startup (the 2 rotation buffers keep their zeros; each
            # row-tile's DMA only rewrites rows 0-10). Zero weights x
            # finite garbage would be fine, but zero x zero avoids any
            # NaN-pattern risk.
            # w0 and the first two x tiles arrive pre-zero-padded from the
            # host, so these DMAs are dependency-free and stay at the head
            # of their rings; the two xr pool buffers keep rows 11-127
            # zero forever (later row-tiles only rewrite rows 0-10).
            w0r = cpool.tile([128, 1024], BF16, name="w0r", tag="w0r")
            nc.scalar.dma_start(out=w0r[:], in_=w0_d[:])

            xr0 = xpool.tile([128, NT], BF16, name="xr", tag="xr")
            nc.sync.dma_start(out=xr0[:], in_=x0p_d[:, 0:NT])

            w1r = [cpool.tile([128, 1024], BF16, name=f"w1r{kc}",
                              tag=f"w1r{kc}") for kc in range(NCH)]
            w2r = [cpool.tile([128, 512], BF16, name=f"w2r{kc}",
                              tag=f"w2r{kc}") for kc in range(NCH)]
            w3r = [cpool.tile([128, 256], BF16, name=f"w3r{kc}",
                              tag=f"w3r{kc}") for kc in range(NCH)]
            # w1 in 128KB half-chunks (lo: out cols 0-511 feeding mc 0-3
            # of wave kc, hi: cols 512-1023), spread over all three DMA
            # queues in wave-consumption order so rt0's waves are PE-bound
            # from ~9us. The scalar queue's DMAs (its queue blocks on each
            # transfer) all land before its first ACT is needed (~23us);
            # sync and gpsimd carry the rest.
            def w1lo(kc, eng):
                eng.dma_start(out=w1r[kc][:, 0:512], in_=w1_d[kc][:, 0:512])

            def w1hi(kc, eng):
                eng.dma_start(out=w1r[kc][:, 512:1024],
                              in_=w1_d[kc][:, 512:1024])

            # lo halves stream on sync, hi halves on gpsimd, one pair per
            # wave; the scalar queue carries nothing (each queued DMA
            # blocks it for the transfer time and the first sin is needed
            # at ~12us). Small/late tensors follow the w1 stream.
            for kc in range(NCH):
                w1lo(kc, nc.sync)
            for kc in range(NCH):
                w1hi(kc, nc.gpsimd)
            w4t = cpool.tile([128, 10], F32, name="w4t", tag="w4t")
            nc.gpsimd.dma_start(out=w4t[:], in_=w4_d[:])
            bt = cpool.tile([128, 32], F32, name="bt", tag="bt")
            nc.gpsimd.dma_start(out=bt[:], in_=bias_d[:])
            b4t = cpool.tile([1, 1], F32, name="b4t", tag="b4t")
            nc.gpsimd.dma_start(out=b4t[:], in_=b4_d[:])
            onesr = cpool.tile([128, 128], BF16, name="onesr", tag="onesr")
            nc.gpsimd.dma_start(out=onesr[:], in_=ones_d[:])
            w4b = cpool.tile([128, 1024], BF16, name="w4b", tag="w4b")
            nc.gpsimd.dma_start(out=w4b[:], in_=w4b_d[:])
            # next row-tile's x, then the w2/w3 stream (all needed >20us)
            xr1 = xpool.tile([128, NT], BF16, name="xr", tag="xr")
            nc.sync.dma_start(out=xr1[:], in_=x0p_d[:, NT:2 * NT])
            for kc in (0, 1, 2, 3):
                nc.sync.dma_start(out=w2r[kc][:], in_=w2_d[kc])
            for kc in (4, 5, 6, 7):
                nc.gpsimd.dma_start(out=w2r[kc][:], in_=w2_d[kc])
            for kc in (0, 1, 2, 3, 4, 5):
                nc.gpsimd.dma_start(out=w3r[kc][:], in_=w3_d[kc])
            for kc in (6, 7):
                nc.sync.dma_start(out=w3r[kc][:], in_=w3_d[kc])

            PW = 2 * NT

            # ---- emitters ----------------------------------------------
            def mm_l0(mc, j):
                return dict(lhsT=w0r[:, 128 * mc:128 * (mc + 1)], rhs_idx=0)

            def emit_pair_l0(q, xr, bufs_):
                """L0 pair q: two dense K=128 bf16 matmuls (zero-padded
                weight rows) — same stream shape as every other layer."""
                return emit_pair(1, q, 1, mm_l0, [xr[:]], bufs_)

            def mm_l1(mc, j):
                kc = (mc + j) % NCH
                return dict(lhsT=w1r[kc][:, 128 * mc:128 * (mc + 1)],
                            rhs_idx=kc)

            def mm_l2(mc, j):
                b = mc // 4
                kcl = (mc + j) % 4
                return dict(lhsT=w2r[4 * b + kcl][:, (mc % 4) * 128:
                                                  (mc % 4) * 128 + 128],
                            rhs_idx=4 * b + kcl)

            def mm_l3(mc, j):
                bi = mc // 2
                kcl = (mc + j) % 2
                return dict(lhsT=w3r[2 * bi + kcl][:, (mc % 2) * 128:
                                                   (mc % 2) * 128 + 128],
                            rhs_idx=2 * bi + kcl)

            def emit_pair(lidx, q, nk, mm_args, hin, bufs_):
                """One 2-chunk group: both chunks share one 2-bank PSUM
                tile drained by a single wide Sin."""
                hp = hpool.tile([128, 2 * NT], BF16, name=f"h{lidx}_{q}",
                                tag=f"h{lidx}_{q}", bufs=bufs_)
                pt = ppool.tile([128, PW], F32, name="pt", tag="pt")
                for half in range(2):
                    mc = 2 * q + half
                    dst = pt[:, (half * NT):(half * NT) + NT]
                    for j in range(nk):
                        kw = mm_args(mc, j)
                        kc = kw.pop("rhs_idx")
                        nc.tensor.matmul(dst, rhs=hin[kc],
                                         start=(j == 0),
                                         stop=(j == nk - 1), **kw)
                nc.scalar.activation(hp[:], pt[:], SIN)
                return [hp[:, 0:NT], hp[:, NT:2 * NT]]

            def emit_l1_rt0(h1):
                """Row-tile 0's L1, K-chunk outer: 8 MMs per arriving W1
                chunk, accumulating into 4 simultaneously-held PSUM pairs.
                The PE consumes W1 in DMA arrival order."""
                pts = [ppool.tile([128, PW], F32, name="pt", tag="pt")
                       for _ in range(4)]
                for kc in range(NCH):
                    for mc in range(NCH):  # mc 0-3 need only the lo half
                        nc.tensor.matmul(
                            pts[mc // 2][:, (mc % 2) * NT:(mc % 2 + 1) * NT],
                            lhsT=w1r[kc][:, 128 * mc:128 * (mc + 1)],
                            rhs=h1[kc], start=(kc == 0), stop=(kc == NCH - 1))
                outs = []
                for q in range(4):
                    hp = hpool.tile([128, 2 * NT], BF16, name=f"h2_{q}",
                                    tag=f"h2_{q}", bufs=2)
                    nc.scalar.activation(hp[:], pts[q][:], SIN)
                    outs += [hp[:, 0:NT], hp[:, NT:2 * NT]]
                return outs

            def load_x(rt):
                cs = rt * NT
                xr = xpool.tile([128, NT], BF16, name="xr", tag="xr")
                nc.sync.dma_start(out=xr[0:11, :], in_=xt_d[:, cs:cs + NT])
                return xr

            def flush_tail(pend):
                # partition-reduce of the deferred row-tile's L4 accumulator
                # on the (otherwise idle) gpsimd engine, bias, and store
                p_rt, acc = pend
                red = xpool.tile([128, NT], F32, name="red", tag="red",
                                 bufs=2)
                nc.gpsimd.partition_all_reduce(red[:], acc[:], 128,
                                               bass_isa.ReduceOp.add)
                ot = xpool.tile([1, NT], F32, name="ot", tag="ot")
                nc.vector.tensor_scalar_add(ot[:], red[0:1, :], b4t[:])
                nc.sync.dma_start(out=o_d[p_rt:p_rt + 1, :], in_=ot[0:1, :])

            # ---- generic-biases fallback (v1 structure, unpacked L0) ---
            def emit_layer_generic(lidx, nk, mm_args, hin, bufs_):
                outs = []
                for mc in range(NCH):
                    pt = ppool.tile([128, PW], F32, name="pt", tag="pt")
                    dst = pt[:, 0:NT]
                    for j in range(nk):
                        kw = mm_args(mc, j)
                        kc = kw.pop("rhs_idx")
                        nc.tensor.matmul(dst, rhs=hin[kc],
                                         start=(j == 0),
                                         stop=(j == nk - 1), **kw)
                    h = hpool.tile([128, NT], BF16, name=f"h{lidx}_{mc}",
                                   tag=f"h{lidx}_{mc}", bufs=bufs_)
                    if lidx == 1:
                        nc.scalar.activation(h[:], dst, SIN)
                    else:
                        c = 8 * (lidx - 1) + mc
                        nc.scalar.activation(h[:], dst, SIN,
                                             bias=bt[:, c:c + 1])
                    outs.append(h[:])
                return outs

            def mm_l0_generic(mc, j):
                return dict(lhsT=w0r[:, 128 * mc:128 * (mc + 1)],
                            rhs_idx=mc)

            if not act_pairs:
                h1 = emit_layer_generic(1, 1, mm_l0_generic,
                                        [xr0[:]] * NCH, 2)
                h2 = emit_layer_generic(2, NCH, mm_l1, h1, 2)
                pend = None
                for rt in range(rt_count):
                    xrn = load_x(rt + 1) if rt + 1 < rt_count else None
                    h3 = emit_layer_generic(3, 4, mm_l2, h2, 1)
                    h1n = (emit_layer_generic(1, 1, mm_l0_generic,
                                              [xrn[:]] * NCH, 2)
                           if xrn is not None else [])
                    h4 = emit_layer_generic(4, 2, mm_l3, h3, 2)
                    h2n = (emit_layer_generic(2, NCH, mm_l1, h1n, 2)
                           if h1n else [])
                    h2 = h2n
                    if pend is not None:
                        flush_tail(pend)
                        pend = None
                    acf = xpool.tile([128, NT], F32, name="acf", tag="acf",
                                     bufs=2)
                    ach = xpool.tile([128, NT], F32, name="ach", tag="ach",
                                     bufs=2)
                    nc.vector.tensor_scalar_mul(acf[:], h4[0],
                                                w4t[:, 0:1])
                    nc.vector.tensor_scalar_mul(ach[:], h4[4],
                                                w4t[:, 4:5])
                    for kc in (1, 2, 3):
                        nc.vector.scalar_tensor_tensor(
                            acf[:], h4[kc], w4t[:, kc:kc + 1],
                            acf[:], mybir.AluOpType.mult,
                            mybir.AluOpType.add)
                    for kc in (5, 6, 7):
                        nc.vector.scalar_tensor_tensor(
                            ach[:], h4[kc], w4t[:, kc:kc + 1],
                            ach[:], mybir.AluOpType.mult,
                            mybir.AluOpType.add)
                    acc = xpool.tile([128, NT], BF16, name="acc", tag="acc",
                                     bufs=2)
                    nc.vector.tensor_tensor(acc[:], acf[:], ach[:],
                                            mybir.AluOpType.add)
                    pend = (rt, acc)
                flush_tail(pend)
            else:
                # ---- merged steady-state schedule ----------------------
                h1 = []
                for q in range(4):
                    h1 += emit_pair_l0(q, xr0, 2)
                h2 = emit_l1_rt0(h1)
                xrn = xr1
                pend = None
                for rt in range(rt_count):
                    last = rt == rt_count - 1
                    h3, h4, h1n = [None] * 8, [None] * 8, []
                    if last:
                        acf = ach = None
                    else:
                        acf = xpool.tile([128, NT], F32, name="acf",
                                         tag="acf", bufs=2)
                        ach = xpool.tile([128, NT], F32, name="ach",
                                         tag="ach", bufs=2)

                    def l1q(q):
                        h2[2 * q:2 * q + 2] = emit_pair(2, q, NCH, mm_l1,
                                                        h1, 2)

                    def l2q(q):
                        h3[2 * q:2 * q + 2] = emit_pair(3, q, 4, mm_l2,
                                                        h2, 1)

                    def l3q(q):
                        h4[2 * q:2 * q + 2] = emit_pair(4, q, 2, mm_l3,
                                                        h3, 2)
                        if last:
                            return  # L4 runs as a PE burst after l3q(3)
                        dstt = acf if q < 2 else ach
                        for mc in (2 * q, 2 * q + 1):
                            if mc % 4 == 0:
                                nc.vector.tensor_scalar_mul(
                                    dstt[:], h4[mc],
                                    w4t[:, mc:mc + 1])
                            else:
                                nc.vector.scalar_tensor_tensor(
                                    dstt[:], h4[mc],
                                    w4t[:, mc:mc + 1], dstt[:],
                                    mybir.AluOpType.mult,
                                    mybir.AluOpType.add)

                    def l0p(q):
                        if xrn is not None:
                            h1n.extend(emit_pair_l0(q, xrn, 2))

                    # L1(rt) rebuilds h2 in place: l2q(0)/l2q(1) read only
                    # chunks 0-3 (block 0), written by l1q(0)/l1q(1) above
                    # them; l2q(2)/l2q(3) read 4-7, written by l1q(2)/(3).
                    if rt > 0:
                        l1q(0)
                        l1q(1)
                        if pend is not None:
                            flush_tail(pend)
                            pend = None
                        l1q(2)
                    l2q(0)
                    l0p(0)
                    l2q(1)
                    l3q(0)
                    if rt > 0:
                        l1q(3)
                    l0p(1)
                    l3q(1)
                    l2q(2)
                    l2q(3)
                    l0p(2)
                    l3q(2)
                    l0p(3)
                    l3q(3)

                    if not last:
                        acc = xpool.tile([128, NT], F32, name="acc",
                                         tag="acc", bufs=2)
                        nc.vector.tensor_tensor(acc[:], acf[:], ach[:],
                                                mybir.AluOpType.add)
                        pend = (rt, acc)
                    h1 = h1n
                    xrn = load_x(rt + 2) if rt + 2 < rt_count else None

                # last row-tile's L4 as a short PE burst (accumulating
                # w4b partition-reduce matmuls) so no serial DVE chain is
                # exposed at the very end, then +b4 and store
                ptl = ppool.tile([128, PW], F32, name="ptl", tag="pt")
                for mc in range(NCH):
                    nc.tensor.matmul(ptl[:, 0:NT],
                                     lhsT=w4b[:, 128 * mc:128 * (mc + 1)],
                                     rhs=h4[mc],
                                     start=(mc == 0), stop=(mc == NCH - 1))
                ot = xpool.tile([1, NT], F32, name="ot", tag="ot")
                nc.vector.tensor_scalar_add(ot[:], ptl[0:1, 0:NT], b4t[:])
                nc.sync.dma_start(out=o_d[rt_count - 1:rt_count, :],
                                  in_=ot[0:1, :])

    nc.compile()
    return nc


def _get_program(act_pairs):
    key = act_pairs
    if key not in _PROGRAMS:
        _PROGRAMS[key] = _build_program(act_pairs=act_pairs)
    return _PROGRAMS[key]


def _rne11(x):
    """fp32 -> float32r grid: round-to-nearest-even keeping 11 mantissa bits
    (verified bit-identical to the on-chip f32r CAST)."""
    u = np.ascontiguousarray(x, np.float32).view(np.uint32).astype(np.uint64)
    bias = ((u >> 12) & 1) + (1 << 11) - 1
    return (((u + bias) >> 12) << 12).astype(np.uint32).view(np.float32)


def kernel(X, lb_X, ub_X, W0, b0, W1, b1, W2, b2, W3, b3, W4, b4):
    X = np.asarray(X, np.float32)
    lb = np.asarray(lb_X, np.float64)
    ub = np.asarray(ub_X, np.float64)
    W0 = np.asarray(W0, np.float64)
    b0 = np.asarray(b0, np.float64)

    # fold input normalization h = X*s + t into W0/b0:
    #   sin((X*s+t)@W0 + b0) = sin(X@(s[:,None]*W0) + (t@W0 + b0))
    # then expand K to 11 bf16 rows for full precision in bf16 matmuls:
    #   z = xh@W0h + xh@W0l + xl@W0h + bias_hi + bias_lo
    s = 2.0 / (ub - lb)
    t = -2.0 * lb / (ub - lb) - 1.0
    b0p = (b0 + t @ W0).astype(np.float32).reshape(1024)
    sW0 = (s[:, None] * W0).astype(np.float32)
    W0h = sW0.astype(_BF16NP)
    W0l = (sW0 - W0h.astype(np.float32)).astype(_BF16NP)
    bh = b0p.astype(_BF16NP)
    bl = (b0p - bh.astype(np.float32)).astype(_BF16NP)
    W0p = np.zeros((128, 1024), _BF16NP)
    W0p[0:3] = W0h
    W0p[3:6] = W0l
    W0p[6:9] = W0h
    W0p[9] = bh
    W0p[10] = bl

    W1 = np.asarray(W1, np.float32)
    W2 = np.asarray(W2, np.float32)
    W3 = np.asarray(W3, np.float32)
    W4 = np.asarray(W4, np.float32)
    b1 = np.asarray(b1, np.float32).reshape(1024)
    b2 = np.asarray(b2, np.float32).reshape(1024)
    b3 = np.asarray(b3, np.float32).reshape(1024)

    w1h = np.ascontiguousarray(W1.reshape(8, 128, 1024)).astype(_BF16NP)
    # W2: 2 blocks of 512x512 -> [4b+kcl] = W2[512b+128kcl:+128, 512b:+512]
    w2h = np.zeros((8, 128, 512), np.float32)
    for b in range(2):
        for kcl in range(4):
            w2h[4 * b + kcl] = W2[512 * b + 128 * kcl:512 * b + 128 * (kcl + 1),
                                  512 * b:512 * (b + 1)]
    # W3: 4 blocks of 256x256 -> [2bi+kcl] = W3[256bi+128kcl:+128, 256bi:+256]
    w3h = np.zeros((8, 128, 256), np.float32)
    for bi in range(4):
        for kcl in range(2):
            w3h[2 * bi + kcl] = W3[256 * bi + 128 * kcl:256 * bi + 128 * (kcl + 1),
                                   256 * bi:256 * (bi + 1)]
    # W4 [1024,1] -> [128,10]: col kc = W4[128kc:+128, 0]; cols 8-9 = ones
    # (stationary operand of the f32r partition-reduce matmul)
    w4h = np.ones((128, 10), np.float32)
    w4h[:, :8] = W4.reshape(8, 128).T
    # hidden-layer biases [128, 8] chunk-major columns (layers 1-3; layer 0's
    # bias is folded into the W0 ones-row)
    bh = np.zeros((128, 32), np.float32)
    for i, bb in enumerate([b1, b2, b3], start=1):
        bh[:, 8 * i:8 * (i + 1)] = bb.reshape(8, 128).T
    b4h = np.asarray(b4, np.float32).reshape(1, 1)

    w2h = w2h.astype(_BF16NP)
    w3h = w3h.astype(_BF16NP)
    # partition-reduce stationaries, zero-padded to full 128-wide weights
    # so the reduce matmuls keep the dense LDW/MM pipeline shape
    onesh = np.zeros((128, 128), _BF16NP)
    onesh[:, 0] = 1
    w4bp = np.zeros((128, 1024), np.float32)
    for mc in range(8):
        w4bp[:, 128 * mc] = w4h[:, mc]
    w4bp = w4bp.astype(_BF16NP)
    act_pairs = not (b1.any() or b2.any() or b3.any())
    nc = _get_program(act_pairs)

    in_maps = []
    for c in range(N_CORES):
        xt = np.ones((11, R), _BF16NP)  # rows 9-10 = ones (bias rows)
        xc = X[c * R:(c + 1) * R].T
        xch = xc.astype(_BF16NP)
        xt[0:3] = xch
        xt[3:6] = xch
        xt[6:9] = (xc - xch.astype(np.float32)).astype(_BF16NP)
        x0p = np.zeros((128, 1024), _BF16NP)  # first 2 row-tiles, padded
        x0p[0:11] = xt[:, 0:1024]
        in_maps.append({
            "xt": xt, "w0": W0p, "w1": w1h, "w2": w2h, "w3": w3h,
            "w4": w4h, "bias": bh, "b4": b4h,
            "onesr": onesh, "w4b": w4bp, "x0p": x0p,
        })

    trace = bool(int(os.environ.get("KERNEL_TRACE", "0")))
    res = run_bass_kernel_spmd(nc, in_maps, list(range(N_CORES)), trace=trace)
    global LAST_RESULTS
    LAST_RESULTS = res

    out = np.concatenate([res.results[c]["o"].reshape(R) for c in range(N_CORES)])
    return out.reshape(N_FULL, 1).astype(np.float32)


# revision 20
# speedup vs baseline: 1.0392x; 1.0157x over previous
"""BsPINN forward MLP on 8 TRN2 NeuronCores (Bass/Tile), data-parallel over rows.

Network (per reference):
  h = 2*(X-lb)/(ub-lb)-1          [N,3]   (folded into W0/b0 on host)
  h = sin(h @ W0 + b0)            [N,1024]
  h = sin(h @ W1 + b1)            [N,1024] dense
  h = sin(h @ (W2*m2) + b2)       [N,1024] block-diag 2x(512x512)
  h = sin(h @ (W3*m3) + b3)       [N,1024] block-diag 4x(256x256)
  out = h @ W4 + b4               [N,1]

Design notes (v2; v1 measured 960us, see kernel_baseline.py):
  * Activations kept feature-major on chip (hT: features->partitions,
    rows->free); out_chunkT = W_chunk.T @ hT via nc.tensor.matmul, moving
    free dim 512 (one PSUM bank). Matmuls in float32r (fp32 RNE-rounded to
    11 mantissa bits on host), streaming 1 cycle/row.
  * Block-diagonal masks exploited by multiplying only in-block K-chunks
    (L2: 4 of 8, L3: 2 of 8) -- 60.3 GFLOP/core instead of 103.
  * L0 (K=4) thin matmuls are packed 2-per-PSUM-pair into DIFFERENT 32-row
    groups (tile_position (0,0) and (32,0), with x and W0 replicated at
    partition 32): the row-group LDWEIGHTS of the second MM overlaps the
    first MM's stream and the two MMs run concurrently, ~330ns per pair
    instead of 2x320ns serialized (v1 trace: thin LDW cannot hide behind a
    full-array MM).
  * Single merged per-row-tile schedule instead of v1's two phases, using
    the block-diag locality (L3 pair q needs only L2 pair q's output):
    [L1q0 L1q1 L1q2 L2q0 L0'p0 L2q1 L1q3 L0'p1 L2q2 L3q0 L0'p2 L2q3 L0'p3
     L3q1 L3q2 L3q3] keeps the ACT(sin) queue's duty even (~68%) so PSUM
    pair rotation never blocks the PE (v1's phase A was ACT-saturated).
  * Row-tile 0's L1 runs K-chunk-outer "waves" (8 MMs per arriving W1
    chunk, accumulating into 4 held PSUM pairs) so the PE consumes W1
    strictly in DMA arrival order; W1 chunks are spread over the scalar/
    sync HWDGE queues + the gpsimd SWDGE queue. Steady state starts ~20us
    vs ~34us in v1.
  * Sin on the scalar engine drains two PSUM banks per instruction; zero
    hidden biases (true here) let the bias ride W0's ones-row; a
    bias-general fallback program is compiled if biases are nonzero.
  * L4 (1024->1) runs as DVE per-partition multiply-accumulate plus a
    single f32r ones-matmul partition reduce, deferred one row-tile.
"""
import os
import numpy as np
import ml_dtypes

_BF16NP = ml_dtypes.bfloat16

try:  # run_bass_kernel_spmd(trace=True) imports this; absent in some images
    from antenv import axon_hooks as _axon_hooks  # noqa: F401
except ImportError:
    import sys
    import types
    _m = types.ModuleType("antenv.axon_hooks")
    _hook = [None]
    _m.set_axon_ntff_profile_hook = lambda h: _hook.__setitem__(0, h)
    _m.get_axon_ntff_profile_hook = lambda: _hook[0]
    sys.modules["antenv.axon_hooks"] = _m

import concourse.bass as bass
import concourse.tile as tile
from concourse import bacc, mybir
from concourse.bass_utils import run_bass_kernel_spmd
from concourse import bass_isa

N_CORES = 8
N_FULL = 131072
R = N_FULL // N_CORES          # 16384 rows per core
NT = 512                       # matmul moving free dim (one PSUM bank, fp32)
RT = R // NT                   # 32 row tiles per core
NCH = 8                        # feature chunks (1024 / 128)

F32 = mybir.dt.float32
F32R = mybir.dt.float32r
BF16 = mybir.dt.bfloat16
SIN = mybir.ActivationFunctionType.Sin
IDENT = mybir.ActivationFunctionType.Identity

LAST_RESULTS = None
_PROGRAMS = {}


def _build_program(rt_count=RT, n_cores=N_CORES, act_pairs=True):
    nc = bacc.Bacc("TRN2", target_bir_lowering=False, debug=False,
                   num_devices=n_cores)

    xt_d = nc.dram_tensor("xt", [11, R], BF16, kind="ExternalInput").ap()
    w0_d = nc.dram_tensor("w0", [128, 1024], BF16, kind="ExternalInput").ap()
    x0p_d = nc.dram_tensor("x0p", [128, 2 * NT], BF16, kind="ExternalInput").ap()
    w1_d = nc.dram_tensor("w1", [8, 128, 1024], BF16, kind="ExternalInput").ap()
    w2_d = nc.dram_tensor("w2", [8, 128, 512], BF16, kind="ExternalInput").ap()
    w3_d = nc.dram_tensor("w3", [8, 128, 256], BF16, kind="ExternalInput").ap()
    w4_d = nc.dram_tensor("w4", [128, 10], F32, kind="ExternalInput").ap()
    bias_d = nc.dram_tensor("bias", [128, 32], F32, kind="ExternalInput").ap()
    b4_d = nc.dram_tensor("b4", [1, 1], F32, kind="ExternalInput").ap()
    ones_d = nc.dram_tensor("onesr", [128, 128], BF16, kind="ExternalInput").ap()
    w4b_d = nc.dram_tensor("w4b", [128, 1024], BF16, kind="ExternalInput").ap()
    o_d = nc.dram_tensor("o", [RT, NT], F32, kind="ExternalOutput").ap()

    with tile.TileContext(nc) as tc:
        with (
            tc.tile_pool(name="const", bufs=1) as cpool,
            tc.tile_pool(name="hbuf", bufs=2) as hpool,
            tc.tile_pool(name="xio", bufs=2) as xpool,
            tc.tile_pool(name="psum", bufs=4, space="PSUM") as ppool,
        ):
            # ---- one-time weight/bias setup ----------------------------
            # Weights arrive host-pre-rounded to the f32r grid and DMA
            # straight into f32r tiles. W0 (with its replica at partition
            # 32 for the row-group-packed L0) leads the scalar queue so
            # L0(rt0) can run at ~1us; W1 chunks are consumed in strict
            # arrival order by rt0's K-outer waves, spread over scalar/
            # sync/gpsimd so all 8 land by ~11us.
            # W0 row 3 holds b0 (the rhs carries a matching ones-row).
            # L0 runs as plain dense K=128 bf16 matmuls: weight rows 11-127
            # are zero, and the x tiles' rows 11-127 are zeroed once at
            # startup (the 2 rotation buffers keep their zeros; each
            # row-tile's DMA only rewrites rows 0-10). Zero weights x
            # finite garbage would be fine, but zero x zero avoids any
            # NaN-pattern risk.
            # w0 and the first two x tiles arrive pre-zero-padded from the
            # host, so these DMAs are dependency-free and stay at the head
            # of their rings; the two xr pool buffers keep rows 11-127
            # zero forever (later row-tiles only rewrite rows 0-10).
            w0r = cpool.tile([128, 1024], BF16, name="w0r", tag="w0r")
            nc.scalar.dma_start(out=w0r[:, 0:256], in_=w0_d[:, 0:256])
            nc.scalar.dma_start(out=w0r[:, 256:1024], in_=w0_d[:, 256:1024])

            xr0 = xpool.tile([128, NT], BF16, name="xr", tag="xr")
            nc.sync.dma_start(out=xr0[:], in_=x0p_d[:, 0:NT])

            w1r = [cpool.tile([128, 1024], BF16, name=f"w1r{kc}",
                              tag=f"w1r{kc}") for kc in range(NCH)]
            w2r = [cpool.tile([128, 512], BF16, name=f"w2r{kc}",
                              tag=f"w2r{kc}") for kc in range(NCH)]
            w3r = [cpool.tile([128, 256], BF16, name=f"w3r{kc}",
                              tag=f"w3r{kc}") for kc in range(NCH)]
            # w1 in 128KB half-chunks (lo: out cols 0-511 feeding mc 0-3
            # of wave kc, hi: cols 512-1023), spread over all three DMA
            # queues in wave-consumption order so rt0's waves are PE-bound
            # from ~9us. The scalar queue's DMAs (its queue blocks on each
            # transfer) all land before its first ACT is needed (~23us);
            # sync and gpsimd carry the rest.
            def w1lo(kc, eng):
                eng.dma_start(out=w1r[kc][:, 0:512], in_=w1_d[kc][:, 0:512])

            def w1hi(kc, eng):
                eng.dma_start(out=w1r[kc][:, 512:1024],
                              in_=w1_d[kc][:, 512:1024])

            # lo halves stream on sync, hi halves on gpsimd, one pair per
            # wave; the scalar queue carries nothing (each queued DMA
            # blocks it for the transfer time and the first sin is needed
            # at ~12us). Small/late tensors follow the w1 stream.
            for kc in range(NCH):
                w1lo(kc, nc.sync)
            for kc in range(NCH):
                w1hi(kc, nc.gpsimd)
            w4t = cpool.tile([128, 10], F32, name="w4t", tag="w4t")
            nc.gpsimd.dma_start(out=w4t[:], in_=w4_d[:])
            bt = cpool.tile([128, 32], F32, name="bt", tag="bt")
            nc.gpsimd.dma_start(out=bt[:], in_=bias_d[:])
            b4t = cpool.tile([1, 1], F32, name="b4t", tag="b4t")
            nc.gpsimd.dma_start(out=b4t[:], in_=b4_d[:])
            onesr = cpool.tile([128, 128], BF16, name="onesr", tag="onesr")
            nc.gpsimd.dma_start(out=onesr[:], in_=ones_d[:])
            w4b = cpool.tile([128, 1024], BF16, name="w4b", tag="w4b")
            nc.gpsimd.dma_start(out=w4b[:], in_=w4b_d[:])
            # next row-tile's x, then the w2/w3 stream (all needed >20us)
            xr1 = xpool.tile([128, NT], BF16, name="xr", tag="xr")
            nc.sync.dma_start(out=xr1[:], in_=x0p_d[:, NT:2 * NT])
            for kc in (0, 1, 2, 3):
                nc.sync.dma_start(out=w2r[kc][:], in_=w2_d[kc])
            for kc in (4, 5, 6, 7):
                nc.gpsimd.dma_start(out=w2r[kc][:], in_=w2_d[kc])
            for kc in (0, 1, 2, 3, 4, 5):
                nc.gpsimd.dma_start(out=w3r[kc][:], in_=w3_d[kc])
            for kc in (6, 7):
                nc.sync.dma_start(out=w3r[kc][:], in_=w3_d[kc])

            PW = 2 * NT

            # ---- emitters ----------------------------------------------
            def emit_wave_l0(w, xr, bufs_):
                """L0 wave w: four K=11 matmuls packed into the four 32-row
                groups of the PE array (x and W0 replicated per band), all
                streaming concurrently into 4 PSUM banks (2 pairs)."""
                hps, pts = [], []
                for p in range(2):
                    hps.append(hpool.tile([128, 2 * NT], BF16,
                                          name=f"h1_{2 * w + p}",
                                          tag=f"h1_{2 * w + p}", bufs=bufs_))
                    pts.append(ppool.tile([128, PW], F32, name="pt",
                                          tag="pt"))
                for c in range(4):
                    mc = 4 * w + c
                    rows = slice(32 * c, 32 * c + 11)
                    nc.tensor.matmul(
                        pts[c // 2][:, (c % 2) * NT:(c % 2 + 1) * NT],
                        lhsT=w0r[rows, 128 * mc:128 * (mc + 1)],
                        rhs=xr[rows, :], start=True, stop=True,
                        tile_position=(32 * c, 0))
                for p in range(2):
                    nc.scalar.activation(hps[p][:], pts[p][:], SIN)
                return [hps[0][:, 0:NT], hps[0][:, NT:2 * NT],
                        hps[1][:, 0:NT], hps[1][:, NT:2 * NT]]

            def mm_l1(mc, j):
                kc = (mc + j) % NCH
                return dict(lhsT=w1r[kc][:, 128 * mc:128 * (mc + 1)],
                            rhs_idx=kc)

            def mm_l2(mc, j):
                b = mc // 4
                kcl = (mc + j) % 4
                return dict(lhsT=w2r[4 * b + kcl][:, (mc % 4) * 128:
                                                  (mc % 4) * 128 + 128],
                            rhs_idx=4 * b + kcl)

            def mm_l3(mc, j):
                bi = mc // 2
                kcl = (mc + j) % 2
                return dict(lhsT=w3r[2 * bi + kcl][:, (mc % 2) * 128:
                                                   (mc % 2) * 128 + 128],
                            rhs_idx=2 * bi + kcl)

            def emit_pair(lidx, q, nk, mm_args, hin, bufs_):
                """One 2-chunk group: both chunks share one 2-bank PSUM
                tile drained by a single wide Sin."""
                hp = hpool.tile([128, 2 * NT], BF16, name=f"h{lidx}_{q}",
                                tag=f"h{lidx}_{q}", bufs=bufs_)
                pt = ppool.tile([128, PW], F32, name="pt", tag="pt")
                for half in range(2):
                    mc = 2 * q + half
                    dst = pt[:, (half * NT):(half * NT) + NT]
                    for j in range(nk):
                        kw = mm_args(mc, j)
                        kc = kw.pop("rhs_idx")
                        nc.tensor.matmul(dst, rhs=hin[kc],
                                         start=(j == 0),
                                         stop=(j == nk - 1), **kw)
                nc.scalar.activation(hp[:], pt[:], SIN)
                return [hp[:, 0:NT], hp[:, NT:2 * NT]]

            def emit_l1_rt0(h1):
                """Row-tile 0's L1, K-chunk outer: 8 MMs per arriving W1
                chunk, accumulating into 4 simultaneously-held PSUM pairs.
                The PE consumes W1 in DMA arrival order."""
                pts = [ppool.tile([128, PW], F32, name="pt", tag="pt")
                       for _ in range(4)]
                for kc in range(NCH):
                    for mc in range(NCH):  # mc 0-3 need only the lo half
                        nc.tensor.matmul(
                            pts[mc // 2][:, (mc % 2) * NT:(mc % 2 + 1) * NT],
                            lhsT=w1r[kc][:, 128 * mc:128 * (mc + 1)],
                            rhs=h1[kc], start=(kc == 0), stop=(kc == NCH - 1))
                outs = []
                for q in range(4):
                    hp = hpool.tile([128, 2 * NT], BF16, name=f"h2_{q}",
                                    tag=f"h2_{q}", bufs=2)
                    nc.scalar.activation(hp[:], pts[q][:], SIN)
                    outs += [hp[:, 0:NT], hp[:, NT:2 * NT]]
                return outs

            def load_x(rt):
                cs = rt * NT
                xr = xpool.tile([128, NT], BF16, name="xr", tag="xr")
                for c in range(4):
                    nc.sync.dma_start(out=xr[32 * c:32 * c + 11, :],
                                      in_=xt_d[:, cs:cs + NT])
                return xr

            def flush_tail(pend):
                # partition-reduce of the deferred row-tile's L4 accumulator
                # on the (otherwise idle) gpsimd engine, bias, and store
                p_rt, acc = pend
                red = xpool.tile([128, NT], F32, name="red", tag="red",
                                 bufs=2)
                nc.gpsimd.partition_all_reduce(red[:], acc[:], 128,
                                               bass_isa.ReduceOp.add)
                ot = xpool.tile([1, NT], F32, name="ot", tag="ot")
                nc.vector.tensor_scalar_add(ot[:], red[0:1, :], b4t[:])
                nc.sync.dma_start(out=o_d[p_rt:p_rt + 1, :], in_=ot[0:1, :])

            # ---- generic-biases fallback (v1 structure, unpacked L0) ---
            def emit_layer_generic(lidx, nk, mm_args, hin, bufs_):
                outs = []
                for mc in range(NCH):
                    pt = ppool.tile([128, PW], F32, name="pt", tag="pt")
                    dst = pt[:, 0:NT]
                    for j in range(nk):
                        kw = mm_args(mc, j)
                        kc = kw.pop("rhs_idx")
                        nc.tensor.matmul(dst, rhs=hin[kc],
                                         start=(j == 0),
                                         stop=(j == nk - 1), **kw)
                    h = hpool.tile([128, NT], BF16, name=f"h{lidx}_{mc}",
                                   tag=f"h{lidx}_{mc}", bufs=bufs_)
                    if lidx == 1:
                        nc.scalar.activation(h[:], dst, SIN)
                    else:
                        c = 8 * (lidx - 1) + mc
                        nc.scalar.activation(h[:], dst, SIN,
                                             bias=bt[:, c:c + 1])
                    outs.append(h[:])
                return outs

            def mm_l0_generic(mc, j):
                # band 0 only (w0r/xr hold replicated bands for the packed
                # main path; summing all 128 rows would count x four times)
                return dict(lhsT=w0r[0:11, 128 * mc:128 * (mc + 1)],
                            rhs_idx=mc)

            if not act_pairs:
                h1 = emit_layer_generic(1, 1, mm_l0_generic,
                                        [xr0[0:11, :]] * NCH, 2)
                h2 = emit_layer_generic(2, NCH, mm_l1, h1, 2)
                pend = None
                for rt in range(rt_count):
                    xrn = load_x(rt + 1) if rt + 1 < rt_count else None
                    h3 = emit_layer_generic(3, 4, mm_l2, h2, 1)
                    h1n = (emit_layer_generic(1, 1, mm_l0_generic,
                                              [xrn[0:11, :]] * NCH, 2)
                           if xrn is not None else [])
                    h4 = emit_layer_generic(4, 2, mm_l3, h3, 2)
                    h2n = (emit_layer_generic(2, NCH, mm_l1, h1n, 2)
                           if h1n else [])
                    h2 = h2n
                    if pend is not None:
                        flush_tail(pend)
                        pend = None
                    acf = xpool.tile([128, NT], F32, name="acf", tag="acf",
                                     bufs=2)
                    ach = xpool.tile([128, NT], F32, name="ach", tag="ach",
                                     bufs=2)
                    nc.vector.tensor_scalar_mul(acf[:], h4[0],
                                                w4t[:, 0:1])
                    nc.vector.tensor_scalar_mul(ach[:], h4[4],
                                                w4t[:, 4:5])
                    for kc in (1, 2, 3):
                        nc.vector.scalar_tensor_tensor(
                            acf[:], h4[kc], w4t[:, kc:kc + 1],
                            acf[:], mybir.AluOpType.mult,
                            mybir.AluOpType.add)
                    for kc in (5, 6, 7):
                        nc.vector.scalar_tensor_tensor(
                            ach[:], h4[kc], w4t[:, kc:kc + 1],
                            ach[:], mybir.AluOpType.mult,
                            mybir.AluOpType.add)
                    acc = xpool.tile([128, NT], BF16, name="acc", tag="acc",
                                     bufs=2)
                    nc.vector.tensor_tensor(acc[:], acf[:], ach[:],
                                            mybir.AluOpType.add)
                    pend = (rt, acc)
                flush_tail(pend)
            else:
                # ---- merged steady-state schedule ----------------------
                h1 = []
                for w in range(2):
                    h1 += emit_wave_l0(w, xr0, 2)
                h2 = emit_l1_rt0(h1)
                xrn = xr1
                pend = None
                for rt in range(rt_count):
                    last = rt == rt_count - 1
                    h3, h4, h1n = [None] * 8, [None] * 8, []
                    if last:
                        acf = ach = None
                    else:
                        acf = xpool.tile([128, NT], F32, name="acf",
                                         tag="acf", bufs=2)
                        ach = xpool.tile([128, NT], F32, name="ach",
                                         tag="ach", bufs=2)

                    def l1q(q):
                        h2[2 * q:2 * q + 2] = emit_pair(2, q, NCH, mm_l1,
                                                        h1, 2)

                    def l2q(q):
                        h3[2 * q:2 * q + 2] = emit_pair(3, q, 4, mm_l2,
                                                        h2, 1)

                    def l3q(q):
                        h4[2 * q:2 * q + 2] = emit_pair(4, q, 2, mm_l3,
                                                        h3, 2)
                        if last:
                            return  # L4 runs as a PE burst after l3q(3)
                        dstt = acf if q < 2 else ach
                        for mc in (2 * q, 2 * q + 1):
                            if mc % 4 == 0:
                                nc.vector.tensor_scalar_mul(
                                    dstt[:], h4[mc],
                                    w4t[:, mc:mc + 1])
                            else:
                                nc.vector.scalar_tensor_tensor(
                                    dstt[:], h4[mc],
                                    w4t[:, mc:mc + 1], dstt[:],
                                    mybir.AluOpType.mult,
                                    mybir.AluOpType.add)

                    def l0p(w):
                        if xrn is not None and w in (0, 2):
                            h1n.extend(emit_wave_l0(w // 2, xrn, 2))

                    # L1(rt) rebuilds h2 in place: l2q(0)/l2q(1) read only
                    # chunks 0-3 (block 0), written by l1q(0)/l1q(1) above
                    # them; l2q(2)/l2q(3) read 4-7, written by l1q(2)/(3).
                    if rt > 0:
                        l1q(0)
                        l1q(1)
                        if pend is not None:
                            flush_tail(pend)
                            pend = None
                        l1q(2)
                    l2q(0)
                    l0p(0)
                    l2q(1)
                    l3q(0)
                    if rt > 0:
                        l1q(3)
                    l0p(1)
                    l3q(1)
                    l2q(2)
                    l2q(3)
                    l0p(2)
                    l3q(2)
                    l0p(3)
                    l3q(3)

                    if not last:
                        acc = xpool.tile([128, NT], F32, name="acc",
                                         tag="acc", bufs=2)
                        nc.vector.tensor_tensor(acc[:], acf[:], ach[:],
                                                mybir.AluOpType.add)
                        pend = (rt, acc)
                    h1 = h1n
                    xrn = load_x(rt + 2) if rt + 2 < rt_count else None

                # last row-tile's L4 as a short PE burst (accumulating
                # w4b partition-reduce matmuls) so no serial DVE chain is
                # exposed at the very end, then +b4 and store
                ptl = ppool.tile([128, PW], F32, name="ptl", tag="pt")
                for mc in range(NCH):
                    nc.tensor.matmul(ptl[:, 0:NT],
                                     lhsT=w4b[:, 128 * mc:128 * (mc + 1)],
                                     rhs=h4[mc],
                                     start=(mc == 0), stop=(mc == NCH - 1))
                ot = xpool.tile([1, NT], F32, name="ot", tag="ot")
                nc.vector.tensor_scalar_add(ot[:], ptl[0:1, 0:NT], b4t[:])
                nc.sync.dma_start(out=o_d[rt_count - 1:rt_count, :],
                                  in_=ot[0:1, :])

    nc.compile()
    return nc


def _get_program(act_pairs):
    key = act_pairs
    if key not in _PROGRAMS:
        _PROGRAMS[key] = _build_program(act_pairs=act_pairs)
    return _PROGRAMS[key]


def _rne11(x):
    """fp32 -> float32r grid: round-to-nearest-even keeping 11 mantissa bits
    (verified bit-identical to the on-chip f32r CAST)."""
    u = np.ascontiguousarray(x, np.float32).view(np.uint32).astype(np.uint64)
    bias = ((u >> 12) & 1) + (1 << 11) - 1
    return (((u + bias) >> 12) << 12).astype(np.uint32).view(np.float32)


def kernel(X, lb_X, ub_X, W0, b0, W1, b1, W2, b2, W3, b3, W4, b4):
    X = np.asarray(X, np.float32)
    lb = np.asarray(lb_X, np.float64)
    ub = np.asarray(ub_X, np.float64)
    W0 = np.asarray(W0, np.float64)
    b0 = np.asarray(b0, np.float64)

    # fold input normalization h = X*s + t into W0/b0:
    #   sin((X*s+t)@W0 + b0) = sin(X@(s[:,None]*W0) + (t@W0 + b0))
    # then expand K to 11 bf16 rows for full precision in bf16 matmuls:
    #   z = xh@W0h + xh@W0l + xl@W0h + bias_hi + bias_lo
    s = 2.0 / (ub - lb)
    t = -2.0 * lb / (ub - lb) - 1.0
    b0p = (b0 + t @ W0).astype(np.float32).reshape(1024)
    sW0 = (s[:, None] * W0).astype(np.float32)
    W0h = sW0.astype(_BF16NP)
    W0l = (sW0 - W0h.astype(np.float32)).astype(_BF16NP)
    bh = b0p.astype(_BF16NP)
    bl = (b0p - bh.astype(np.float32)).astype(_BF16NP)
    W0p = np.zeros((128, 1024), _BF16NP)
    for c in range(4):
        W0p[32 * c + 0:32 * c + 3] = W0h
        W0p[32 * c + 3:32 * c + 6] = W0l
        W0p[32 * c + 6:32 * c + 9] = W0h
        W0p[32 * c + 9] = bh
        W0p[32 * c + 10] = bl

    W1 = np.asarray(W1, np.float32)
    W2 = np.asarray(W2, np.float32)
    W3 = np.asarray(W3, np.float32)
    W4 = np.asarray(W4, np.float32)
    b1 = np.asarray(b1, np.float32).reshape(1024)
    b2 = np.asarray(b2, np.float32).reshape(1024)
    b3 = np.asarray(b3, np.float32).reshape(1024)

    w1h = np.ascontiguousarray(W1.reshape(8, 128, 1024)).astype(_BF16NP)
    # W2: 2 blocks of 512x512 -> [4b+kcl] = W2[512b+128kcl:+128, 512b:+512]
    w2h = np.zeros((8, 128, 512), np.float32)
    for b in range(2):
        for kcl in range(4):
            w2h[4 * b + kcl] = W2[512 * b + 128 * kcl:512 * b + 128 * (kcl + 1),
                                  512 * b:512 * (b + 1)]
    # W3: 4 blocks of 256x256 -> [2bi+kcl] = W3[256bi+128kcl:+128, 256bi:+256]
    w3h = np.zeros((8, 128, 256), np.float32)
    for bi in range(4):
        for kcl in range(2):
            w3h[2 * bi + kcl] = W3[256 * bi + 128 * kcl:256 * bi + 128 * (kcl + 1),
                                   256 * bi:256 * (bi + 1)]
    # W4 [1024,1] -> [128,10]: col kc = W4[128kc:+128, 0]; cols 8-9 = ones
    # (stationary operand of the f32r partition-reduce matmul)
    w4h = np.ones((128, 10), np.float32)
    w4h[:, :8] = W4.reshape(8, 128).T
    # hidden-layer biases [128, 8] chunk-major columns (layers 1-3; layer 0's
    # bias is folded into the W0 ones-row)
    bh = np.zeros((128, 32), np.float32)
    for i, bb in enumerate([b1, b2, b3], start=1):
        bh[:, 8 * i:8 * (i + 1)] = bb.reshape(8, 128).T
    b4h = np.asarray(b4, np.float32).reshape(1, 1)

    w2h = w2h.astype(_BF16NP)
    w3h = w3h.astype(_BF16NP)
    # partition-reduce stationaries, zero-padded to full 128-wide weights
    # so the reduce matmuls keep the dense LDW/MM pipeline shape
    onesh = np.zeros((128, 128), _BF16NP)
    onesh[:, 0] = 1
    w4bp = np.zeros((128, 1024), np.float32)
    for mc in range(8):
        w4bp[:, 128 * mc] = w4h[:, mc]
    w4bp = w4bp.astype(_BF16NP)
    act_pairs = not (b1.any() or b2.any() or b3.any())
    nc = _get_program(act_pairs)

    in_maps = []
    for c in range(N_CORES):
        xt = np.ones((11, R), _BF16NP)  # rows 9-10 = ones (bias rows)
        xc = X[c * R:(c + 1) * R].T
        xch = xc.astype(_BF16NP)
        xt[0:3] = xch
        xt[3:6] = xch
        xt[6:9] = (xc - xch.astype(np.float32)).astype(_BF16NP)
        x0p = np.zeros((128, 1024), _BF16NP)  # first 2 row-tiles, padded
        for c in range(4):
            x0p[32 * c:32 * c + 11] = xt[:, 0:1024]
        in_maps.append({
            "xt": xt, "w0": W0p, "w1": w1h, "w2": w2h, "w3": w3h,
            "w4": w4h, "bias": bh, "b4": b4h,
            "onesr": onesh, "w4b": w4bp, "x0p": x0p,
        })

    trace = bool(int(os.environ.get("KERNEL_TRACE", "0")))
    res = run_bass_kernel_spmd(nc, in_maps, list(range(N_CORES)), trace=trace)
    global LAST_RESULTS
    LAST_RESULTS = res

    out = np.concatenate([res.results[c]["o"].reshape(R) for c in range(N_CORES)])
    return out.reshape(N_FULL, 1).astype(np.float32)
